# revision 2
# baseline (speedup 1.0000x reference)
"""Trainium2 Bass kernel for nn_GTCNN (product-graph GTCNN, 2 layers, K collapsed).

Math (per batch b, x: [M=8192, 32]):
  Adj = s0*I + s1*kron(I_t, As) + s2*kron(At, I_s) + s3*kron(At, As),  T=64, N=128
  h0 = x @ W1 + b1
  h_{l+1} = tanh((Adj @ h_l) @ Heff_l),   Heff_l = sum_k H[l, k]   (einsum collapses k)
  out = h2 @ W2 + b2

Device algorithm per layer (the three mixes commute):
  u = At-mix(z) over t;  Zpre = P(n-mix) z + Q(n-mix) u;  z' = tanh(Zpre @ Heff)
  with P = s0*I + s1*As, Q = s2*I + s3*As folded on host.

Sharding: core c -> (b = c // 4, t-quarter q = c % 4). Layer 1 computed fully per
b (4x redundant, no collectives); layer 2 + output restricted to the 16-t quarter.

Layouts (n = 32*nh + nl, t = 32*c + tl):
  NM  [n, t*32 + h]                      node-on-partition (As / P / Q matmuls)
  FD  [32*nh + h,  t*32 + nl]            feature-on-partition (W1/Heff/W2 matmuls,
                                         block-diag kron(I4, W) stationaries)
  FDT [32*nh + tl, c*1024 + nl*32 + h]   t-on-partition (At matmuls, stationaries
                                         kron(I4, At 32x32 block), PSUM-accum over c)
Layout moves are DVE 32x32 StreamTranspose ops whose in/out access patterns
steer which (block, within-block) geometry is transposed.
"""

import numpy as np

T, NS, B, FIN, HID, FOUT = 64, 128, 2, 32, 32, 16
M = T * NS
NCORES, NQ = 8, 4
TQ = T // NQ  # 16 t's per quarter

_CACHE = {}


def _build_nc():
    from contextlib import ExitStack

    import concourse.mybir as mybir
    import concourse.tile as tile
    from concourse import bacc
    from concourse.bass import ds

    fp = mybir.dt.float32
    AF = mybir.ActivationFunctionType

    nc = bacc.Bacc(
        "TRN2",
        target_bir_lowering=False,
        debug=False,
        enable_asserts=False,
        num_devices=NCORES,
    )

    xb = nc.dram_tensor("xb", [M, FIN], fp, kind="ExternalInput")
    w1i4 = nc.dram_tensor("w1i4", [128, 128], fp, kind="ExternalInput")
    b1t = nc.dram_tensor("b1t", [128, 1], fp, kind="ExternalInput")
    atbd = nc.dram_tensor("atbd", [2, 2, 128, 128], fp, kind="ExternalInput")
    atbq = nc.dram_tensor("atbq", [2, 128, 128], fp, kind="ExternalInput")
    pq = nc.dram_tensor("pq", [2, 128, 128], fp, kind="ExternalInput")
    hi4 = nc.dram_tensor("hi4", [2, 128, 128], fp, kind="ExternalInput")
    w2i4 = nc.dram_tensor("w2i4", [128, 128], fp, kind="ExternalInput")
    b2t = nc.dram_tensor("b2t", [128, 1], fp, kind="ExternalInput")
    outb = nc.dram_tensor("outb", [TQ * NS, FOUT], fp, kind="ExternalOutput")

    C512 = [slice(512 * j, 512 * (j + 1)) for j in range(4)]

    with tile.TileContext(nc) as tc, ExitStack() as ctx:
        const = ctx.enter_context(tc.tile_pool(name="const", bufs=1))
        st = ctx.enter_context(tc.tile_pool(name="st", bufs=1))
        ps = ctx.enter_context(tc.tile_pool(name="ps", bufs=2, space="PSUM"))

        # Core id -> quarter offset register (used for the layer-2 column slice).
        pid = nc.vector.partition_id()
        toff = (pid % NQ) * TQ  # t-offset of this core's quarter

        # ---- constants to SBUF ----
        w1i4_s = const.tile([128, 128], fp, tag="w1i4")
        nc.sync.dma_start(w1i4_s[:], w1i4.ap())
        b1t_s = const.tile([128, 1], fp, tag="b1t")
        nc.sync.dma_start(b1t_s[:], b1t.ap())
        atbd_s = const.tile([128, 4, 128], fp, tag="atbd")
        nc.sync.dma_start(atbd_s[:], atbd.ap().rearrange("a b p c -> p (a b) c"))
        atbq_s = const.tile([128, 2, 128], fp, tag="atbq")
        nc.sync.dma_start(atbq_s[:], atbq.ap().rearrange("a p c -> p a c"))
        pq_s = const.tile([128, 2, 128], fp, tag="pq")
        nc.sync.dma_start(pq_s[:], pq.ap().rearrange("a p c -> p a c"))
        hi4_s = const.tile([128, 2, 128], fp, tag="hi4")
        nc.sync.dma_start(hi4_s[:], hi4.ap().rearrange("a p c -> p a c"))
        w2i4_s = const.tile([128, 128], fp, tag="w2i4")
        nc.sync.dma_start(w2i4_s[:], w2i4.ap())
        b2t_s = const.tile([128, 1], fp, tag="b2t")
        nc.sync.dma_start(b2t_s[:], b2t.ap())
        pmat = pq_s[:, 0, :]
        qmat = pq_s[:, 1, :]

        # ---- x load: NM [n, (t, f)] ----
        x_nm = st.tile([128, 2048], fp, tag="x_nm")
        xv = xb.ap().rearrange("(t n) f -> n t f", n=128)
        x_nm_v = x_nm[:].rearrange("p (t f) -> p t f", f=32)
        for j in range(4):
            nc.sync.dma_start(x_nm_v[:, 16 * j : 16 * (j + 1), :], xv[:, 16 * j : 16 * (j + 1), :])

        # ---- x -> FD ----
        x_fd = st.tile([128, 2048], fp, tag="x_fd")
        for j in range(4):
            nc.vector.transpose(out=x_fd[:, C512[j]], in_=x_nm[:, C512[j]])

        # ---- h0 = x @ W1 + b1 (FD-out) ----
        h0pre = ps.tile([128, 2048], fp, tag="big")
        for j in range(4):
            nc.tensor.matmul(h0pre[:, C512[j]], w1i4_s[:], x_fd[:, C512[j]], start=True, stop=True)
        h0_fd = st.tile([128, 2048], fp, tag="h0_fd")
        for j in range(4):
            nc.scalar.activation(h0_fd[:, C512[j]], h0pre[:, C512[j]], AF.Identity, bias=b1t_s[:, 0:1])

        def t_and_n_mix(z_fd, g_tag, full):
            """From z in FD layout produce (z_nm or None, u contributions).

            Returns (g_fdt tile, z_nm tile) for the full path; layer 2 slices are
            handled by the caller."""
            g = st.tile([128, 2048], fp, tag=g_tag)
            gi = z_fd[:].rearrange("p (c tl nl) -> p c nl tl", c=2, tl=32, nl=32)
            go = g[:].rearrange("p (c nl h) -> p c nl h", c=2, nl=32, h=32)
            for c in range(2):
                for k in range(2):
                    nc.vector.transpose(
                        out=go[:, c, 16 * k : 16 * (k + 1), :], in_=gi[:, c, 16 * k : 16 * (k + 1), :]
                    )
            return g

        # =========================== layer 1 (full) ===========================
        g1 = t_and_n_mix(h0_fd, "g", True)
        z_nm = st.tile([128, 2048], fp, tag="z_nm")
        for j in range(4):
            nc.vector.transpose(out=z_nm[:, C512[j]], in_=h0_fd[:, C512[j]])

        u_ps = ps.tile([128, 2048], fp, tag="big")
        g1r = g1[:].rearrange("p (c nl h) -> p c h nl", c=2, nl=32, h=32)
        for cp in range(2):
            for hh in range(2):
                for c in range(2):
                    nc.tensor.matmul(
                        u_ps[:, cp * 1024 + 512 * hh : cp * 1024 + 512 * (hh + 1)],
                        atbd_s[:, 2 * c + cp, :],
                        g1r[:, c, 16 * hh : 16 * (hh + 1), :],
                        start=(c == 0),
                        stop=(c == 1),
                    )

        u_nm = st.tile([128, 2048], fp, tag="u_nm")
        ui = u_ps[:].rearrange("p (cp h nl) -> p cp h nl", cp=2, h=32, nl=32)
        uo = u_nm[:].rearrange("p (cp tl h) -> p cp h tl", cp=2, tl=32, h=32)
        for cp in range(2):
            for k in range(2):
                nc.vector.transpose(
                    out=uo[:, cp, 16 * k : 16 * (k + 1), :], in_=ui[:, cp, 16 * k : 16 * (k + 1), :]
                )

        zpre = ps.tile([128, 2048], fp, tag="big")
        for j in range(4):
            nc.tensor.matmul(zpre[:, C512[j]], pmat, z_nm[:, C512[j]], start=True, stop=False)
        for j in range(4):
            nc.tensor.matmul(zpre[:, C512[j]], qmat, u_nm[:, C512[j]], start=False, stop=True)

        zf = st.tile([128, 2048], fp, tag="zf")
        for j in range(4):
            nc.vector.transpose(out=zf[:, C512[j]], in_=zpre[:, C512[j]])

        pre1 = ps.tile([128, 2048], fp, tag="big")
        for j in range(4):
            nc.tensor.matmul(pre1[:, C512[j]], hi4_s[:, 0, :], zf[:, C512[j]], start=True, stop=True)
        h1_fd = st.tile([128, 2048], fp, tag="h1_fd")
        for j in range(4):
            nc.scalar.activation(h1_fd[:, C512[j]], pre1[:, C512[j]], AF.Tanh)

        # ====================== layer 2 (t-quarter only) ======================
        g2 = t_and_n_mix(h1_fd, "g", True)  # At-mix needs all t

        z2_nm = st.tile([128, 512], fp, tag="z2_nm")
        h1v = h1_fd[:].rearrange("p (t nl) -> p t nl", nl=32)
        nc.vector.transpose(out=z2_nm[:], in_=h1v[:, ds(toff, TQ), :])

        u2_ps = ps.tile([128, 1024], fp, tag="big")
        g2r = g2[:].rearrange("p (c nl h) -> p c h nl", c=2, nl=32, h=32)
        for hh in range(2):
            for c in range(2):
                nc.tensor.matmul(
                    u2_ps[:, 512 * hh : 512 * (hh + 1)],
                    atbq_s[:, c, :],
                    g2r[:, c, 16 * hh : 16 * (hh + 1), :],
                    start=(c == 0),
                    stop=(c == 1),
                )

        u2_nm = st.tile([128, 1024], fp, tag="u2_nm")
        u2i = u2_ps[:].rearrange("p (h nl) -> p h nl", h=32, nl=32)
        u2o = u2_nm[:].rearrange("p (i h) -> p h i", i=32, h=32)
        for k in range(2):
            nc.vector.transpose(
                out=u2o[:, 16 * k : 16 * (k + 1), :], in_=u2i[:, 16 * k : 16 * (k + 1), :]
            )

        zpre2 = ps.tile([128, 512], fp, tag="big")
        nc.tensor.matmul(zpre2[:], pmat, z2_nm[:], start=True, stop=False)
        nc.tensor.matmul(zpre2[:], qmat, u2_nm[:, 0:512], start=False, stop=True)

        z2f = st.tile([128, 512], fp, tag="z2f")
        nc.vector.transpose(out=z2f[:], in_=zpre2[:])

        pre2 = ps.tile([128, 512], fp, tag="big")
        nc.tensor.matmul(pre2[:], hi4_s[:, 1, :], z2f[:], start=True, stop=True)
        h2_fd = st.tile([128, 512], fp, tag="h2_fd")
        nc.scalar.activation(h2_fd[:], pre2[:], AF.Tanh)

        opre = ps.tile([128, 512], fp, tag="big")
        nc.tensor.matmul(opre[:], w2i4_s[:], h2_fd[:], start=True, stop=True)
        out_fd = st.tile([128, 512], fp, tag="out_fd")
        nc.scalar.activation(out_fd[:], opre[:], AF.Identity, bias=b2t_s[:, 0:1])

        out_nm = st.tile([128, 512], fp, tag="out_nm")
        nc.vector.transpose(out=out_nm[:], in_=out_fd[:])

        onv = out_nm[:].rearrange("p (i j2) -> p i j2", j2=32)
        ov = outb.ap().rearrange("(i n) j -> n i j", n=128)
        nc.sync.dma_start(ov, onv[:, :, 0:FOUT])

    nc.compile()
    return nc


def _host_weights(Adj_t, Adj_s, s, H, W1, b1, W2, b2):
    f4 = np.float32
    I4 = np.eye(4, dtype=f4)
    I128 = np.eye(128, dtype=f4)
    Heff = H.sum(axis=1).astype(f4)  # [2, 32, 32]

    P = (s[0] * I128 + s[1] * Adj_s).astype(f4)
    Q = (s[2] * I128 + s[3] * Adj_s).astype(f4)
    pq = np.stack([P, Q])

    w1i4 = np.kron(I4, W1.astype(f4))
    hi4 = np.stack([np.kron(I4, Heff[0]), np.kron(I4, Heff[1])])
    w2pad = np.zeros((32, 32), dtype=f4)
    w2pad[:, :FOUT] = W2
    w2i4 = np.kron(I4, w2pad)

    b1t = np.tile(b1.astype(f4), 4)[:, None]
    b2pad = np.zeros(32, dtype=f4)
    b2pad[:FOUT] = b2
    b2t = np.tile(b2pad, 4)[:, None]

    atbd = np.zeros((2, 2, 128, 128), dtype=f4)
    for c in range(2):
        for cp in range(2):
            atbd[c, cp] = np.kron(I4, Adj_t[32 * c : 32 * (c + 1), 32 * cp : 32 * (cp + 1)].astype(f4))

    atbq_all = np.zeros((NQ, 2, 128, 128), dtype=f4)
    for q in range(NQ):
        for c in range(2):
            blk = np.zeros((32, 32), dtype=f4)
            blk[:, :TQ] = Adj_t[32 * c : 32 * (c + 1), TQ * q : TQ * (q + 1)]
            atbq_all[q, c] = np.kron(I4, blk)

    return dict(w1i4=w1i4, b1t=b1t, atbd=atbd, pq=pq, hi4=hi4, w2i4=w2i4, b2t=b2t), atbq_all


def _in_maps(inputs):
    f4 = np.float32
    x = np.ascontiguousarray(np.asarray(inputs["x"], dtype=f4))
    shared, atbq_all = _host_weights(
        np.asarray(inputs["Adj_t"], dtype=f4),
        np.asarray(inputs["Adj_s"], dtype=f4),
        np.asarray(inputs["s"], dtype=f4),
        np.asarray(inputs["H"], dtype=f4),
        np.asarray(inputs["W1"], dtype=f4),
        np.asarray(inputs["b1"], dtype=f4),
        np.asarray(inputs["W2"], dtype=f4),
        np.asarray(inputs["b2"], dtype=f4),
    )
    maps = []
    for c in range(NCORES):
        b, q = c // NQ, c % NQ
        m = {"xb": np.ascontiguousarray(x[b]), "atbq": np.ascontiguousarray(atbq_all[q])}
        m.update({k: np.ascontiguousarray(v) for k, v in shared.items()})
        maps.append(m)
    return maps


def kernel(**inputs) -> np.ndarray:
    from concourse import bass_utils

    if "nc" not in _CACHE:
        _CACHE["nc"] = _build_nc()
    nc = _CACHE["nc"]

    maps = _in_maps(inputs)
    import os

    trace = bool(int(os.environ.get("GTCNN_TRACE", "0")))
    res = bass_utils.run_bass_kernel_spmd(
        nc,
        maps,
        core_ids=list(range(NCORES)),
        trace=trace,
        trace_cores=list(range(NCORES)) if trace else None,
        stitch_traces=False,
    )
    _CACHE["last_results"] = res

    out = np.empty((B, M, FOUT), dtype=np.float32)
    for c in range(NCORES):
        b, q = c // NQ, c % NQ
        out[b, 2048 * q : 2048 * (q + 1), :] = res.results[c]["outb"]
    return out



# revision 6
# speedup vs baseline: 1.5176x; 1.5176x over previous
"""Trainium2 Bass kernel for nn_GTCNN (product-graph GTCNN, 2 layers, K collapsed).

Math (per batch b, x: [M=8192, 32]):
  Adj = s0*I + s1*kron(I_t, As) + s2*kron(At, I_s) + s3*kron(At, As),  T=64, N=128
  h0 = x @ W1 + b1
  h_{l+1} = tanh((Adj @ h_l) @ Heff_l),   Heff_l = sum_k H[l, k]   (einsum collapses k)
  out = h2 @ W2 + b2

Device algorithm (the feature mix commutes with the node mixes):
  layer 1 "H-first", Heff0 folded into W1 on the host:
    w0 = x @ (W1 Heff0) + b1 Heff0
    z1 = tanh(P w0 + Q At w0)          P = s0*I + s1*As, Q = s2*I + s3*As
  layer 2 "H-last", quarter only:
    v  = P z1 + Q At z1                (t-quarter rows)
    out = tanh(v Heff1) @ W2 + b2
  (At, As symmetric -> they serve directly as matmul stationaries.)

Sharding: core c -> (b = c // 4, t-quarter q = c % 4). Layer 1 computed fully per
b (4x redundant, no collectives); layer 2 + output restricted to the 16-t quarter.

Layouts (n = 32*nh + nl, t = 32*c + tl, partition-block always nh):
  FD   [32*nh + h,  (c, tl, nl)]    feature-on-partition (W1/Heff/W2 matmuls)
  NM   [n, (c, tl, h)]              node-on-partition, t-major (P/Q "w" side)
  NM'  [n, (c, h, tl)]              node-on-partition, h-major (P/Q result, z1)
  FDT  [32*nh + tl, (c, h-or-nl, ...)]  t-on-partition (At matmuls)
All layout moves are DVE 32x32 StreamTranspose with CONTIGUOUS writes (strided
writes measured 4.6 ns/elem vs 1.17 contiguous); the unavoidable strided side
is always a read (1.8x) or a strided-inner matmul moving operand.

Perf design:
  - fp16 on-chip (1 cycle/col matmuls, fast weight load); PSUM stays fp32;
    rel err ~1e-3, tolerance is 2e-2
  - every PSUM->SBUF crossing is one ACT op (bias / copy / tanh) doing the
    fp32->fp16 conversion
  - x pre-marshalled on host into FD (one contiguous DMA); out stored FD
  - 8 warmup matmuls + dummy matmuls with data deps on mid-pipeline tiles
    keep the PE busy so the HAM clock gate never re-throttles to 1.2 GHz;
    dummies write (start=stop=True) into PSUM regions whose real writers
    also open with start=True, so they are overwritten harmlessly
"""

import numpy as np

T, NS, B, FIN, HID, FOUT = 64, 128, 2, 32, 32, 16
M = T * NS
NCORES, NQ = 8, 4
TQ = T // NQ  # 16 t's per quarter

_CACHE = {}

# column-block offsets (x128) inside the packed const tensor
_W1, _P, _Q, _H1, _W2 = 0, 1, 2, 3, 4
_ATBD = 5  # 4 blocks: 2*c + cp
_ATBQ = 9  # 2 blocks: c
_NCBLK = 11


def _build_nc():
    from contextlib import ExitStack

    import concourse.mybir as mybir
    import concourse.tile as tile
    from concourse import bacc
    from concourse.bass import ds

    fp = mybir.dt.float32
    f16 = mybir.dt.float16
    AF = mybir.ActivationFunctionType

    nc = bacc.Bacc(
        "TRN2",
        target_bir_lowering=False,
        debug=False,
        enable_asserts=False,
        num_devices=NCORES,
    )

    xfd_d = nc.dram_tensor("xfd", [128, 2048], f16, kind="ExternalInput")
    cst_d = nc.dram_tensor("cst", [128, _NCBLK * 128], f16, kind="ExternalInput")
    bias_d = nc.dram_tensor("bias", [128, 2], fp, kind="ExternalInput")
    outfd_d = nc.dram_tensor("outfd", [128, 512], fp, kind="ExternalOutput")

    C512 = [slice(512 * j, 512 * (j + 1)) for j in range(4)]
    H1024 = [slice(1024 * j, 1024 * (j + 1)) for j in range(2)]

    with tile.TileContext(nc) as tc, ExitStack() as ctx:
        const = ctx.enter_context(tc.tile_pool(name="const", bufs=1))
        st = ctx.enter_context(tc.tile_pool(name="st", bufs=1))
        ps = ctx.enter_context(tc.tile_pool(name="ps", bufs=2, space="PSUM"))

        pid = nc.tensor.partition_id()  # on PE: consumed by the vpre matmul AP
        cq = (pid % NQ) // 2  # which t-half holds this core's quarter
        tl0 = (pid % 2) * TQ  # tl offset inside that half

        # ---- PE warmup scratch: memset on gpsimd (otherwise-idle engine) ----
        warm_sb = st.tile([128, 512], f16, tag="warm_sb")
        nc.gpsimd.memset(warm_sb[:], 0.0)

        # ---- loads: x (contiguous FD layout), packed consts, fp32 biases ----
        x_fd = st.tile([128, 2048], f16, tag="x_fd")
        nc.sync.dma_start(x_fd[:], xfd_d.ap())
        cs = const.tile([128, _NCBLK * 128], f16, tag="cs")
        nc.sync.dma_start(cs[:], cst_d.ap())
        bt = const.tile([128, 2], fp, tag="bt")
        nc.sync.dma_start(bt[:], bias_d.ap())

        def blk(i):
            return cs[:, 128 * i : 128 * (i + 1)]

        w1m, pmat, qmat, h1m, w2m = blk(_W1), blk(_P), blk(_Q), blk(_H1), blk(_W2)
        b1t = bt[:, 0:1]
        b2t = bt[:, 1:2]
        mm = nc.tensor.matmul

        def dummy(out, src):
            """Keep-warm matmul: garbage into a region whose real writer
            opens with start=True. Fires when `src` (SBUF fp16) is ready."""
            mm(out, src[:, 0:128], src[:, 0:512], start=True, stop=True,
               skip_group_check=True)

        # ---- PE warmup: ~8 x 512-col matmuls keep the PE busy ~3.5us while
        # the DMAs land, so HAM un-throttles the clock before the real work.
        w0pre = ps.tile([128, 2048], fp, tag="big")
        for _ in range(8):
            dummy(w0pre[:, C512[0]], warm_sb)

        # =========================== layer 1 (full) ===========================
        # w0 = x @ W1' + b1'   -> FD [h-part, (c, tl, nl)]
        for j in range(4):
            mm(w0pre[:, C512[j]], w1m, x_fd[:, C512[j]], start=True, stop=True)
        w0_fd = st.tile([128, 2048], f16, tag="w0_fd")
        for j in range(2):
            nc.scalar.activation(w0_fd[:, H1024[j]], w0pre[:, H1024[j]], AF.Identity, bias=b1t)

        # g0 = FDT of w0 [tl-part, (c, nl, h)]: strided read, contiguous write
        g0 = st.tile([128, 2048], f16, tag="g0")
        gi = w0_fd[:].rearrange("p (c tl nl) -> p c nl tl", c=2, tl=32, nl=32)
        go = g0[:].rearrange("p (c nl h) -> p c nl h", c=2, nl=32, h=32)
        u0_ps = ps.tile([128, 2048], fp, tag="big")
        dummy(u0_ps[:, C512[0]], w0_fd)  # fires mid ACT/transpose phase
        for c in range(2):
            nc.vector.transpose(out=go[:, c], in_=gi[:, c])
        # w0_nm [n, (c, tl, h)]: contiguous both sides
        w0_nm = st.tile([128, 2048], f16, tag="w0_nm")
        for j in range(2):
            nc.vector.transpose(out=w0_nm[:, H1024[j]], in_=w0_fd[:, H1024[j]])

        # u0 = At-mix(w0): FDT, PSUM-accum over c -> free (cp, h, nl)
        dummy(u0_ps[:, C512[0]], g0)
        g0r = g0[:].rearrange("p (c nl h) -> p c h nl", c=2, nl=32, h=32)
        for cp in range(2):
            for hh in range(2):
                for c in range(2):
                    mm(
                        u0_ps[:, cp * 1024 + 512 * hh : cp * 1024 + 512 * (hh + 1)],
                        blk(_ATBD + 2 * c + cp),
                        g0r[:, c, 16 * hh : 16 * (hh + 1), :],
                        start=(c == 0),
                        stop=(c == 1),
                    )
        u0_sb = st.tile([128, 2048], f16, tag="u0_sb")
        for j in range(2):
            nc.scalar.activation(u0_sb[:, H1024[j]], u0_ps[:, H1024[j]], AF.Identity)

        # u0_nm [n, (cp, h, tl')]: contiguous both sides
        u0_nm = st.tile([128, 2048], f16, tag="u0_nm")
        for j in range(2):
            nc.vector.transpose(out=u0_nm[:, H1024[j]], in_=u0_sb[:, H1024[j]])

        # z1 = tanh(P w0 + Q u0)  -> NM' [n, (c, h, tl)]
        # P moving: w0_nm viewed (c, h, tl) = strided-inner; Q moving: contiguous
        zpre0 = ps.tile([128, 2048], fp, tag="big")
        dummy(zpre0[:, C512[0]], u0_sb)
        w0v = w0_nm[:].rearrange("p (c tl h) -> p c h tl", c=2, tl=32, h=32)
        for c in range(2):
            for hh in range(2):
                j = 2 * c + hh
                mm(zpre0[:, C512[j]], pmat, w0v[:, c, 16 * hh : 16 * (hh + 1), :],
                   start=True, stop=False)
        for j in range(4):
            mm(zpre0[:, C512[j]], qmat, u0_nm[:, C512[j]], start=False, stop=True)
        z1_nm = st.tile([128, 2048], f16, tag="z1_nm")
        for j in range(2):
            nc.scalar.activation(z1_nm[:, H1024[j]], zpre0[:, H1024[j]], AF.Tanh)

        # ====================== layer 2 (t-quarter only) ======================
        # g1 = FDT of z1 [tl-part, (c, h, nl)]: contiguous both sides
        g1 = st.tile([128, 2048], f16, tag="g1")
        u1_ps = ps.tile([128, 1024], fp, tag="big")
        dummy(u1_ps[:, 0:512], z1_nm)
        for j in range(2):
            nc.vector.transpose(out=g1[:, H1024[j]], in_=z1_nm[:, H1024[j]])

        # u1 = At[quarter,:]-mix(z1): PSUM-accum over c -> free (h, nl), part (nh, tq)
        dummy(u1_ps[:, 0:512], g1)
        g1r = g1[:].rearrange("p (c h nl) -> p c h nl", c=2, h=32, nl=32)
        for hh in range(2):
            for c in range(2):
                mm(
                    u1_ps[:, 512 * hh : 512 * (hh + 1)],
                    blk(_ATBQ + c),
                    g1r[:, c, 16 * hh : 16 * (hh + 1), :],
                    start=(c == 0),
                    stop=(c == 1),
                )
        u1_sb = st.tile([128, 1024], f16, tag="u1_sb")
        nc.scalar.activation(u1_sb[:], u1_ps[:], AF.Identity)

        # u1_nm [n, (h, tq32)]: contiguous both sides
        u1_nm = st.tile([128, 1024], f16, tag="u1_nm")
        nc.vector.transpose(out=u1_nm[:], in_=u1_sb[:])

        # v = P z1[quarter] + Q u1  -> NM quarter, free (tq, h)
        vpre = ps.tile([128, 512], fp, tag="big")
        z1v = z1_nm[:].rearrange("p (c h tl) -> p c tl h", c=2, h=32, tl=32)
        u1v = u1_nm[:].rearrange("p (h t) -> p t h", h=32, t=32)
        mm(vpre[:], pmat, z1v[:, ds(cq, 1), ds(tl0, TQ), :], start=True, stop=False)
        mm(vpre[:], qmat, u1v[:, 0:TQ, :], start=False, stop=True)
        v_sb = st.tile([128, 512], f16, tag="v_sb")
        nc.scalar.activation(v_sb[:], vpre[:], AF.Identity)

        # v_fd [h-part, (tq, nl)]: contiguous both sides
        v_fd = st.tile([128, 512], f16, tag="v_fd")
        nc.vector.transpose(out=v_fd[:], in_=v_sb[:])

        # z2 = tanh(v @ H1')  (FD); out = z2 @ W2' + b2  (FD)
        h2pre = ps.tile([128, 512], fp, tag="big")
        mm(h2pre[:], h1m, v_fd[:], start=True, stop=True)
        z2_fd = st.tile([128, 512], f16, tag="z2_fd")
        nc.scalar.activation(z2_fd[:], h2pre[:], AF.Tanh)

        opre = ps.tile([128, 512], fp, tag="big")
        mm(opre[:], w2m, z2_fd[:], start=True, stop=True)
        out_fd = st.tile([128, 512], fp, tag="out_fd")
        nc.scalar.activation(out_fd[:], opre[:], AF.Identity, bias=b2t)

        # store in FD layout; the host unscrambles
        nc.sync.dma_start(outfd_d.ap(), out_fd[:])

    nc.compile()
    return nc


def _host_weights(Adj_t, Adj_s, s, H, W1, b1, W2, b2):
    f4 = np.float32
    I4 = np.eye(4, dtype=f4)
    I128 = np.eye(128, dtype=f4)
    Heff = H.sum(axis=1).astype(f4)  # [2, 32, 32]

    P = (s[0] * I128 + s[1] * Adj_s).astype(f4)
    Q = (s[2] * I128 + s[3] * Adj_s).astype(f4)

    W1p = (W1 @ Heff[0]).astype(f4)  # H-first: fold Heff0 into W1
    b1p = (b1 @ Heff[0]).astype(f4)
    w2pad = np.zeros((32, 32), dtype=f4)
    w2pad[:, :FOUT] = W2

    cst = np.zeros((NQ, 128, _NCBLK * 128), dtype=np.float16)
    for q in range(NQ):
        c = cst[q]
        c[:, 0:128] = np.kron(I4, W1p)
        c[:, 128:256] = P
        c[:, 256:384] = Q
        c[:, 384:512] = np.kron(I4, Heff[1])
        c[:, 512:640] = np.kron(I4, w2pad)
        for cc in range(2):
            for cp in range(2):
                i = _ATBD + 2 * cc + cp
                c[:, 128 * i : 128 * (i + 1)] = np.kron(
                    I4, Adj_t[32 * cc : 32 * (cc + 1), 32 * cp : 32 * (cp + 1)].astype(f4)
                )
        for cc in range(2):
            bq = np.zeros((32, 32), dtype=f4)
            bq[:, :TQ] = Adj_t[32 * cc : 32 * (cc + 1), TQ * q : TQ * (q + 1)]
            i = _ATBQ + cc
            c[:, 128 * i : 128 * (i + 1)] = np.kron(I4, bq)

    bias = np.zeros((128, 2), dtype=f4)
    bias[:, 0] = np.tile(b1p, 4)
    b2pad = np.zeros(32, dtype=f4)
    b2pad[:FOUT] = b2
    bias[:, 1] = np.tile(b2pad, 4)
    return cst, bias


def _in_maps(inputs):
    f4 = np.float32
    x = np.asarray(inputs["x"], dtype=f4)
    cst, bias = _host_weights(
        np.asarray(inputs["Adj_t"], dtype=f4),
        np.asarray(inputs["Adj_s"], dtype=f4),
        np.asarray(inputs["s"], dtype=f4),
        np.asarray(inputs["H"], dtype=f4),
        np.asarray(inputs["W1"], dtype=f4),
        np.asarray(inputs["b1"], dtype=f4),
        np.asarray(inputs["W2"], dtype=f4),
        np.asarray(inputs["b2"], dtype=f4),
    )
    # FD-marshalled x per batch: xfd[32*nh + f, 32*t + nl] = x[b, 128*t + 32*nh + nl, f]
    xfd = [
        np.ascontiguousarray(
            x[b].reshape(T, 4, 32, FIN).transpose(1, 3, 0, 2).reshape(128, 2048)
        ).astype(np.float16)
        for b in range(B)
    ]
    maps = []
    for c in range(NCORES):
        b, q = c // NQ, c % NQ
        maps.append(
            {"xfd": xfd[b], "cst": np.ascontiguousarray(cst[q]), "bias": bias}
        )
    return maps


def kernel(**inputs) -> np.ndarray:
    import os

    from concourse import bass_utils

    if "nc" not in _CACHE:
        _CACHE["nc"] = _build_nc()
    nc = _CACHE["nc"]

    maps = _in_maps(inputs)

    trace = bool(int(os.environ.get("GTCNN_TRACE", "0")))
    res = bass_utils.run_bass_kernel_spmd(
        nc,
        maps,
        core_ids=list(range(NCORES)),
        trace=trace,
        trace_cores=list(range(NCORES)) if trace else None,
        stitch_traces=False,
    )
    _CACHE["last_results"] = res

    out = np.empty((B, M, FOUT), dtype=np.float32)
    for c in range(NCORES):
        b, q = c // NQ, c % NQ
        arr = np.asarray(res.results[c]["outfd"]).reshape(4, 32, TQ, 32)
        out[b, 2048 * q : 2048 * (q + 1), :] = (
            arr[:, :FOUT, :, :].transpose(2, 0, 3, 1).reshape(2048, FOUT)
        )
    return out


# revision 10
# speedup vs baseline: 1.5840x; 1.0437x over previous
"""Trainium2 Bass kernel for nn_GTCNN (product-graph GTCNN, 2 layers, K collapsed).

Math (per batch b, x: [M=8192, 32]):
  Adj = s0*I + s1*kron(I_t, As) + s2*kron(At, I_s) + s3*kron(At, As),  T=64, N=128
  h0 = x @ W1 + b1
  h_{l+1} = tanh((Adj @ h_l) @ Heff_l),   Heff_l = sum_k H[l, k]   (einsum collapses k)
  out = h2 @ W2 + b2

Device algorithm (the feature mix commutes with the node mixes):
  layer 1 "H-first", Heff0 folded into W1 on the host:
    w0 = x @ (W1 Heff0) + b1 Heff0
    z1 = tanh(P w0 + Q At w0)          P = s0*I + s1*As, Q = s2*I + s3*As
  layer 2 "H-last", quarter only:
    v  = P z1 + Q At z1                (t-quarter rows)
    out = tanh(v Heff1) @ W2 + b2
  (At, As symmetric -> they serve directly as matmul stationaries.)

Sharding: core c -> (b = c // 4, t-quarter q = c % 4). Layer 1 computed fully per
b (4x redundant, no collectives); layer 2 + output restricted to the 16-t quarter.

Layouts (n = 32*nh + nl, t = 32*c + tl, partition-block always nh):
  FD   [32*nh + h,  (c, tl, nl)]    feature-on-partition (W1/Heff/W2 matmuls)
  NM   [n, (c, tl, h)]              node-on-partition, t-major (P/Q "w" side)
  NM'  [n, (c, h, tl)]              node-on-partition, h-major (P/Q result, z1)
  FDT  [32*nh + tl, (c, h-or-nl, ...)]  t-on-partition (At matmuls)
All layout moves are DVE 32x32 StreamTranspose with CONTIGUOUS writes (strided
writes measured 4.6 ns/elem vs 1.17 contiguous); the unavoidable strided side
is always a read (1.8x) or a strided-inner matmul moving operand.

Perf design:
  - fp16 on-chip (1 cycle/col matmuls, fast weight load); PSUM stays fp32;
    rel err ~1e-3, tolerance is 2e-2
  - every PSUM->SBUF crossing is one ACT op (bias / copy / tanh) doing the
    fp32->fp16 conversion
  - x pre-marshalled on host into FD (one contiguous DMA); out stored FD
  - 8 warmup matmuls + dummy matmuls with data deps on mid-pipeline tiles
    keep the PE busy so the HAM clock gate never re-throttles to 1.2 GHz;
    dummies write (start=stop=True) into PSUM regions whose real writers
    also open with start=True, so they are overwritten harmlessly
"""

import numpy as np

T, NS, B, FIN, HID, FOUT = 64, 128, 2, 32, 32, 16
M = T * NS
NCORES, NQ = 8, 4
TQ = T // NQ  # 16 t's per quarter

_CACHE = {}

# column-block offsets (x128) inside the packed const tensor
_W1, _P, _Q, _H1, _W2 = 0, 1, 2, 3, 4
_ATBD = 5  # 4 blocks: 2*c + cp
_ATBQ = 9  # 2 blocks: c
_NCBLK = 11


def _build_nc():
    from contextlib import ExitStack

    import concourse.mybir as mybir
    import concourse.tile as tile
    from concourse import bacc
    from concourse.bass import ds

    fp = mybir.dt.float32
    f16 = mybir.dt.float16
    AF = mybir.ActivationFunctionType

    nc = bacc.Bacc(
        "TRN2",
        target_bir_lowering=False,
        debug=False,
        enable_asserts=False,
        num_devices=NCORES,
    )

    xfd_d = nc.dram_tensor("xfd", [128, 2048], f16, kind="ExternalInput")
    cst_d = nc.dram_tensor("cst", [128, _NCBLK * 128], f16, kind="ExternalInput")
    bias_d = nc.dram_tensor("bias", [128, 2], fp, kind="ExternalInput")
    outfd_d = nc.dram_tensor("outfd", [128, 512], fp, kind="ExternalOutput")

    C512 = [slice(512 * j, 512 * (j + 1)) for j in range(4)]
    H1024 = [slice(1024 * j, 1024 * (j + 1)) for j in range(2)]

    with tile.TileContext(nc) as tc, ExitStack() as ctx:
        const = ctx.enter_context(tc.tile_pool(name="const", bufs=1))
        st = ctx.enter_context(tc.tile_pool(name="st", bufs=1))
        ps = ctx.enter_context(tc.tile_pool(name="ps", bufs=2, space="PSUM"))

        pid = nc.tensor.partition_id()  # on PE: consumed by the vpre matmul AP
        cq = (pid % NQ) // 2  # which t-half holds this core's quarter
        tl0 = (pid % 2) * TQ  # tl offset inside that half

        # ---- PE warmup scratch (vector memset: fast, DVE idle this early) ----
        warm_sb = st.tile([128, 512], f16, tag="warm_sb")
        nc.vector.memset(warm_sb[:], 0.0)
        # Preload the tanh activation table off the critical path: without
        # this the first real ACTIVATE pays a lazy 1.3us ACT_TABLE_LOAD.
        tblw = st.tile([128, 1], fp, tag="tblw")
        nc.scalar.activation(tblw[:], warm_sb[:, 0:1], AF.Tanh)

        # ---- loads: x (contiguous FD layout), packed consts, fp32 biases ----
        x_fd = st.tile([128, 2048], f16, tag="x_fd")
        nc.sync.dma_start(x_fd[:], xfd_d.ap())
        cs = const.tile([128, _NCBLK * 128], f16, tag="cs")
        nc.sync.dma_start(cs[:], cst_d.ap())
        bt = const.tile([128, 2], fp, tag="bt")
        nc.sync.dma_start(bt[:], bias_d.ap())

        def blk(i):
            return cs[:, 128 * i : 128 * (i + 1)]

        w1m, pmat, qmat, h1m, w2m = blk(_W1), blk(_P), blk(_Q), blk(_H1), blk(_W2)
        b1t = bt[:, 0:1]
        b2t = bt[:, 1:2]
        mm = nc.tensor.matmul

        def dummy(out, src):
            """Keep-warm matmul: garbage into a region whose real writer
            opens with start=True. Fires when `src` (SBUF fp16) is ready."""
            mm(out, src[:, 0:128], src[:, 0:512], start=True, stop=True,
               skip_group_check=True)

        # ---- PE warmup: ~8 x 512-col matmuls keep the PE busy ~3.5us while
        # the DMAs land, so HAM un-throttles the clock before the real work.
        w0pre = ps.tile([128, 2048], fp, tag="big")
        for _ in range(10):
            dummy(w0pre[:, C512[0]], warm_sb)

        # =========================== layer 1 (full) ===========================
        # w0 = x @ W1' + b1'   -> FD [h-part, (c, tl, nl)]
        for j in range(4):
            mm(w0pre[:, C512[j]], w1m, x_fd[:, C512[j]], start=True, stop=True)
        w0_fd = st.tile([128, 2048], f16, tag="w0_fd")
        for j in range(2):
            nc.scalar.activation(w0_fd[:, H1024[j]], w0pre[:, H1024[j]], AF.Identity, bias=b1t)

        # g0 = FDT of w0 [tl-part, (c, nl, h)]: strided read, contiguous write
        g0 = st.tile([128, 2048], f16, tag="g0")
        gi = w0_fd[:].rearrange("p (c tl nl) -> p c nl tl", c=2, tl=32, nl=32)
        go = g0[:].rearrange("p (c nl h) -> p c nl h", c=2, nl=32, h=32)
        u0_ps = ps.tile([128, 2048], fp, tag="big")
        dummy(u0_ps[:, C512[0]], w0_fd)  # fires mid ACT/transpose phase
        for c in range(2):
            nc.vector.transpose(out=go[:, c], in_=gi[:, c])
        # w0_nm [n, (c, tl, h)]: contiguous both sides
        w0_nm = st.tile([128, 2048], f16, tag="w0_nm")
        for j in range(2):
            nc.vector.transpose(out=w0_nm[:, H1024[j]], in_=w0_fd[:, H1024[j]])

        # u0 = At-mix(w0): FDT, PSUM-accum over c -> free (cp, h, nl)
        dummy(u0_ps[:, C512[0]], g0)
        g0r = g0[:].rearrange("p (c nl h) -> p c h nl", c=2, nl=32, h=32)
        # c outermost: all 4 start-matmuls depend only on g0's first t-half,
        # so they overlap the transpose of the second half
        for c in range(2):
            for cp in range(2):
                for hh in range(2):
                    mm(
                        u0_ps[:, cp * 1024 + 512 * hh : cp * 1024 + 512 * (hh + 1)],
                        blk(_ATBD + 2 * c + cp),
                        g0r[:, c, 16 * hh : 16 * (hh + 1), :],
                        start=(c == 0),
                        stop=(c == 1),
                    )
        u0_sb = st.tile([128, 2048], f16, tag="u0_sb")
        for j in range(2):
            nc.scalar.activation(u0_sb[:, H1024[j]], u0_ps[:, H1024[j]], AF.Identity)

        # u0_nm [n, (cp, h, tl')]: contiguous both sides
        u0_nm = st.tile([128, 2048], f16, tag="u0_nm")
        for j in range(2):
            nc.vector.transpose(out=u0_nm[:, H1024[j]], in_=u0_sb[:, H1024[j]])

        # z1 = tanh(P w0 + Q u0)  -> NM' [n, (c, h, tl)]
        # P moving: w0_nm viewed (c, h, tl) = strided-inner; Q moving: contiguous
        zpre0 = ps.tile([128, 2048], fp, tag="big")
        dummy(zpre0[:, C512[0]], u0_sb)
        w0v = w0_nm[:].rearrange("p (c tl h) -> p c h tl", c=2, tl=32, h=32)
        for c in range(2):
            for hh in range(2):
                j = 2 * c + hh
                mm(zpre0[:, C512[j]], pmat, w0v[:, c, 16 * hh : 16 * (hh + 1), :],
                   start=True, stop=False)
        for j in range(4):
            mm(zpre0[:, C512[j]], qmat, u0_nm[:, C512[j]], start=False, stop=True)
        z1_nm = st.tile([128, 2048], f16, tag="z1_nm")
        for j in range(2):
            nc.scalar.activation(z1_nm[:, H1024[j]], zpre0[:, H1024[j]], AF.Tanh)

        # ====================== layer 2 (t-quarter only) ======================
        # g1 = FDT of z1 [tl-part, (c, h, nl)]: contiguous both sides
        g1 = st.tile([128, 2048], f16, tag="g1")
        u1_ps = ps.tile([128, 1024], fp, tag="big")
        dummy(u1_ps[:, 0:512], z1_nm)
        for j in range(2):
            nc.vector.transpose(out=g1[:, H1024[j]], in_=z1_nm[:, H1024[j]])

        # u1 = At[quarter,:]-mix(z1): PSUM-accum over c -> free (h, nl), part (nh, tq)
        dummy(u1_ps[:, 0:512], g1)
        g1r = g1[:].rearrange("p (c h nl) -> p c h nl", c=2, h=32, nl=32)
        for c in range(2):
            for hh in range(2):
                mm(
                    u1_ps[:, 512 * hh : 512 * (hh + 1)],
                    blk(_ATBQ + c),
                    g1r[:, c, 16 * hh : 16 * (hh + 1), :],
                    start=(c == 0),
                    stop=(c == 1),
                )
        u1_sb = st.tile([128, 1024], f16, tag="u1_sb")
        nc.scalar.activation(u1_sb[:], u1_ps[:], AF.Identity)

        # u1_nm [n, (h, tq32)]: contiguous both sides
        u1_nm = st.tile([128, 1024], f16, tag="u1_nm")
        nc.vector.transpose(out=u1_nm[:], in_=u1_sb[:])

        # v = P z1[quarter] + Q u1  -> NM quarter, free (tq, h)
        vpre = ps.tile([128, 512], fp, tag="big")
        z1v = z1_nm[:].rearrange("p (c h tl) -> p c tl h", c=2, h=32, tl=32)
        u1v = u1_nm[:].rearrange("p (h t) -> p t h", h=32, t=32)
        mm(vpre[:], pmat, z1v[:, ds(cq, 1), ds(tl0, TQ), :], start=True, stop=False)
        mm(vpre[:], qmat, u1v[:, 0:TQ, :], start=False, stop=True)
        v_sb = st.tile([128, 512], f16, tag="v_sb")
        nc.scalar.activation(v_sb[:], vpre[:], AF.Identity)

        # v_fd [h-part, (tq, nl)]: contiguous both sides
        v_fd = st.tile([128, 512], f16, tag="v_fd")
        nc.vector.transpose(out=v_fd[:], in_=v_sb[:])

        # z2 = tanh(v @ H1')  (FD); out = z2 @ W2' + b2  (FD)
        h2pre = ps.tile([128, 512], fp, tag="big")
        mm(h2pre[:], h1m, v_fd[:], start=True, stop=True)
        z2_fd = st.tile([128, 512], f16, tag="z2_fd")
        nc.scalar.activation(z2_fd[:], h2pre[:], AF.Tanh)

        opre = ps.tile([128, 512], fp, tag="big")
        mm(opre[:], w2m, z2_fd[:], start=True, stop=True)
        out_fd = st.tile([128, 512], fp, tag="out_fd")
        nc.scalar.activation(out_fd[:], opre[:], AF.Identity, bias=b2t)

        # store in FD layout; the host unscrambles
        nc.sync.dma_start(outfd_d.ap(), out_fd[:])

    nc.compile()
    return nc


def _host_weights(Adj_t, Adj_s, s, H, W1, b1, W2, b2):
    f4 = np.float32
    I4 = np.eye(4, dtype=f4)
    I128 = np.eye(128, dtype=f4)
    Heff = H.sum(axis=1).astype(f4)  # [2, 32, 32]

    P = (s[0] * I128 + s[1] * Adj_s).astype(f4)
    Q = (s[2] * I128 + s[3] * Adj_s).astype(f4)

    W1p = (W1 @ Heff[0]).astype(f4)  # H-first: fold Heff0 into W1
    b1p = (b1 @ Heff[0]).astype(f4)
    w2pad = np.zeros((32, 32), dtype=f4)
    w2pad[:, :FOUT] = W2

    cst = np.zeros((NQ, 128, _NCBLK * 128), dtype=np.float16)
    for q in range(NQ):
        c = cst[q]
        c[:, 0:128] = np.kron(I4, W1p)
        c[:, 128:256] = P
        c[:, 256:384] = Q
        c[:, 384:512] = np.kron(I4, Heff[1])
        c[:, 512:640] = np.kron(I4, w2pad)
        for cc in range(2):
            for cp in range(2):
                i = _ATBD + 2 * cc + cp
                c[:, 128 * i : 128 * (i + 1)] = np.kron(
                    I4, Adj_t[32 * cc : 32 * (cc + 1), 32 * cp : 32 * (cp + 1)].astype(f4)
                )
        for cc in range(2):
            bq = np.zeros((32, 32), dtype=f4)
            bq[:, :TQ] = Adj_t[32 * cc : 32 * (cc + 1), TQ * q : TQ * (q + 1)]
            i = _ATBQ + cc
            c[:, 128 * i : 128 * (i + 1)] = np.kron(I4, bq)

    bias = np.zeros((128, 2), dtype=f4)
    bias[:, 0] = np.tile(b1p, 4)
    b2pad = np.zeros(32, dtype=f4)
    b2pad[:FOUT] = b2
    bias[:, 1] = np.tile(b2pad, 4)
    return cst, bias


def _in_maps(inputs):
    f4 = np.float32
    x = np.asarray(inputs["x"], dtype=f4)
    cst, bias = _host_weights(
        np.asarray(inputs["Adj_t"], dtype=f4),
        np.asarray(inputs["Adj_s"], dtype=f4),
        np.asarray(inputs["s"], dtype=f4),
        np.asarray(inputs["H"], dtype=f4),
        np.asarray(inputs["W1"], dtype=f4),
        np.asarray(inputs["b1"], dtype=f4),
        np.asarray(inputs["W2"], dtype=f4),
        np.asarray(inputs["b2"], dtype=f4),
    )
    # FD-marshalled x per batch: xfd[32*nh + f, 32*t + nl] = x[b, 128*t + 32*nh + nl, f]
    xfd = [
        np.ascontiguousarray(
            x[b].reshape(T, 4, 32, FIN).transpose(1, 3, 0, 2).reshape(128, 2048)
        ).astype(np.float16)
        for b in range(B)
    ]
    maps = []
    for c in range(NCORES):
        b, q = c // NQ, c % NQ
        maps.append(
            {"xfd": xfd[b], "cst": np.ascontiguousarray(cst[q]), "bias": bias}
        )
    return maps


def kernel(**inputs) -> np.ndarray:
    import os

    from concourse import bass_utils

    if "nc" not in _CACHE:
        _CACHE["nc"] = _build_nc()
    nc = _CACHE["nc"]

    maps = _in_maps(inputs)

    trace = bool(int(os.environ.get("GTCNN_TRACE", "0")))
    res = bass_utils.run_bass_kernel_spmd(
        nc,
        maps,
        core_ids=list(range(NCORES)),
        trace=trace,
        trace_cores=list(range(NCORES)) if trace else None,
        stitch_traces=False,
    )
    _CACHE["last_results"] = res

    out = np.empty((B, M, FOUT), dtype=np.float32)
    for c in range(NCORES):
        b, q = c // NQ, c % NQ
        arr = np.asarray(res.results[c]["outfd"]).reshape(4, 32, TQ, 32)
        out[b, 2048 * q : 2048 * (q + 1), :] = (
            arr[:, :FOUT, :, :].transpose(2, 0, 3, 1).reshape(2048, FOUT)
        )
    return out


# revision 16
# speedup vs baseline: 1.6888x; 1.0662x over previous
"""Trainium2 Bass kernel for nn_GTCNN (product-graph GTCNN, 2 layers, K collapsed).

Math (per batch b, x: [M=8192, 32]):
  Adj = s0*I + s1*kron(I_t, As) + s2*kron(At, I_s) + s3*kron(At, As),  T=64, N=128
  h0 = x @ W1 + b1
  h_{l+1} = tanh((Adj @ h_l) @ Heff_l),   Heff_l = sum_k H[l, k]   (einsum collapses k)
  out = h2 @ W2 + b2

Device algorithm (the feature mix commutes with the node mixes):
  layer 1 "H-first", Heff0 folded into W1 on the host:
    w0 = x @ (W1 Heff0) + b1 Heff0
    z1 = tanh(P w0 + Q At w0)          P = s0*I + s1*As, Q = s2*I + s3*As
  layer 2 "H-last", quarter only:
    v  = P z1 + Q At z1                (t-quarter rows)
    out = tanh(v Heff1) @ W2 + b2
  (At, As symmetric -> they serve directly as matmul stationaries.)

Sharding: core c -> (b = c // 4, t-quarter q = c % 4). Layer 1 computed fully per
b (4x redundant, no collectives); layer 2 + output restricted to the 16-t quarter.

Layouts (n = 32*nh + nl, t = 32*c + tl, partition-block always nh):
  FD   [32*nh + h,  (c, tl, nl)]    feature-on-partition (W1/Heff/W2 matmuls)
  NM   [n, (c, tl, h)]              node-on-partition, t-major (P/Q "w" side)
  NM'  [n, (c, h, tl)]              node-on-partition, h-major (P/Q result, z1)
  FDT  [32*nh + tl, (c, h-or-nl, ...)]  t-on-partition (At matmuls)
All layout moves are DVE 32x32 StreamTranspose with CONTIGUOUS writes (strided
writes measured 4.6 ns/elem vs 1.17 contiguous); the unavoidable strided side
is always a read (1.8x) or a strided-inner matmul moving operand.

Perf design:
  - fp16 on-chip (1 cycle/col matmuls, fast weight load); PSUM stays fp32;
    rel err ~1e-3, tolerance is 2e-2
  - every PSUM->SBUF crossing is one ACT op (bias / copy / tanh) doing the
    fp32->fp16 conversion
  - x pre-marshalled on host into FD (one contiguous DMA); out stored FD
  - 8 warmup matmuls + dummy matmuls with data deps on mid-pipeline tiles
    keep the PE busy so the HAM clock gate never re-throttles to 1.2 GHz;
    dummies write (start=stop=True) into PSUM regions whose real writers
    also open with start=True, so they are overwritten harmlessly
"""

import numpy as np

T, NS, B, FIN, HID, FOUT = 64, 128, 2, 32, 32, 16
M = T * NS
NCORES, NQ = 8, 4
TQ = T // NQ  # 16 t's per quarter

_CACHE = {}

# column-block offsets (x128) inside the packed const tensor
_W1, _P, _Q, _H1, _W2 = 0, 1, 2, 3, 4
_ATBD = 5  # 4 blocks: 2*c + cp
_ATBQ = 9  # 2 blocks: c
_NCBLK = 11


def _build_nc():
    from contextlib import ExitStack

    import concourse.mybir as mybir
    import concourse.tile as tile
    from concourse import bacc
    from concourse.bass import ds

    fp = mybir.dt.float32
    f16 = mybir.dt.float16
    AF = mybir.ActivationFunctionType

    nc = bacc.Bacc(
        "TRN2",
        target_bir_lowering=False,
        debug=False,
        enable_asserts=False,
        num_devices=NCORES,
    )

    xfd_d = nc.dram_tensor("xfd", [128, 2048], f16, kind="ExternalInput")
    cst_d = nc.dram_tensor("cst", [128, _NCBLK * 128], f16, kind="ExternalInput")
    bias_d = nc.dram_tensor("bias", [128, 2], fp, kind="ExternalInput")
    outfd_d = nc.dram_tensor("outfd", [128, 512], fp, kind="ExternalOutput")

    C512 = [slice(512 * j, 512 * (j + 1)) for j in range(4)]
    H1024 = [slice(1024 * j, 1024 * (j + 1)) for j in range(2)]

    with tile.TileContext(nc) as tc, ExitStack() as ctx:
        const = ctx.enter_context(tc.tile_pool(name="const", bufs=1))
        st = ctx.enter_context(tc.tile_pool(name="st", bufs=1))
        ps = ctx.enter_context(tc.tile_pool(name="ps", bufs=2, space="PSUM"))

        pid = nc.tensor.partition_id()  # on PE: consumed by the vpre matmul AP
        cq = (pid % NQ) // 2  # which t-half holds this core's quarter
        tl0 = (pid % 2) * TQ  # tl offset inside that half

        # ---- PE warmup scratch (vector memset: fast, DVE idle this early) ----
        warm_sb = st.tile([128, 512], f16, tag="warm_sb")
        nc.vector.memset(warm_sb[:], 0.0)
        # Preload the tanh activation table off the critical path: without
        # this the first real ACTIVATE pays a lazy 1.3us ACT_TABLE_LOAD.
        tblw = st.tile([128, 1], fp, tag="tblw")
        nc.scalar.activation(tblw[:], warm_sb[:, 0:1], AF.Tanh)

        # ---- loads: consts first (w0pre needs W1'), then x, then biases ----
        cs = const.tile([128, _NCBLK * 128], f16, tag="cs")
        nc.sync.dma_start(cs[:], cst_d.ap())
        x_fd = st.tile([128, 2048], f16, tag="x_fd")
        nc.sync.dma_start(x_fd[:], xfd_d.ap())
        bt = const.tile([128, 2], fp, tag="bt")
        nc.sync.dma_start(bt[:], bias_d.ap())

        def blk(i):
            return cs[:, 128 * i : 128 * (i + 1)]

        w1m, pmat, qmat, h1m, w2m = blk(_W1), blk(_P), blk(_Q), blk(_H1), blk(_W2)
        b1t = bt[:, 0:1]
        b2t = bt[:, 1:2]
        mm = nc.tensor.matmul

        def dummy(out, src):
            """Keep-warm matmul: garbage into a region whose real writer
            opens with start=True. Fires when `src` (SBUF fp16) is ready."""
            mm(out, src[:, 0:128], src[:, 0:512], start=True, stop=True,
               skip_group_check=True)

        # ---- PE warmup: a few matmuls start the HAM clock ramp while the
        # DMAs land. They write the second psum buffer (u0_ps's region), so
        # w0pre has no WAW dependence on them and starts as soon as the DMA
        # semaphores are visible.
        w0pre = ps.tile([128, 2048], fp, tag="big")
        u0_ps = ps.tile([128, 2048], fp, tag="big")
        for _ in range(3):
            dummy(u0_ps[:, C512[0]], warm_sb)

        # =========================== layer 1 (full) ===========================
        # w0 = x @ W1' + b1'   -> FD [h-part, (c, tl, nl)]
        for j in range(4):
            mm(w0pre[:, C512[j]], w1m, x_fd[:, C512[j]], start=True, stop=True)
        w0_fd = st.tile([128, 2048], f16, tag="w0_fd")
        for j in range(2):
            nc.scalar.activation(w0_fd[:, H1024[j]], w0pre[:, H1024[j]], AF.Identity, bias=b1t)

        # g0 = FDT of w0 [tl-part, (c, nl, h)]: strided read, contiguous write
        g0 = st.tile([128, 2048], f16, tag="g0")
        gi = w0_fd[:].rearrange("p (c tl nl) -> p c nl tl", c=2, tl=32, nl=32)
        go = g0[:].rearrange("p (c nl h) -> p c nl h", c=2, nl=32, h=32)
        dummy(u0_ps[:, C512[0]], w0_fd)  # fires mid ACT/transpose phase
        for c in range(2):
            nc.vector.transpose(out=go[:, c], in_=gi[:, c])
        # w0_nm [n, (c, tl, h)]: contiguous both sides
        w0_nm = st.tile([128, 2048], f16, tag="w0_nm")
        for j in range(2):
            nc.vector.transpose(out=w0_nm[:, H1024[j]], in_=w0_fd[:, H1024[j]])

        # u0 = At-mix(w0): FDT, PSUM-accum over c -> free (cp, nl, h).
        # Moving operands are plain contiguous 512-slices of g0 (nl-halves):
        # strided/3D moving APs measured 4 cycles/col vs 1 contiguous.
        # c outermost: all 4 start-matmuls depend only on g0's first t-half,
        # so they overlap the transpose of the second half.
        dummy(u0_ps[:, C512[0]], g0)
        for c in range(2):
            for cp in range(2):
                for nn in range(2):
                    mm(
                        u0_ps[:, cp * 1024 + 512 * nn : cp * 1024 + 512 * (nn + 1)],
                        blk(_ATBD + 2 * c + cp),
                        g0[:, c * 1024 + 512 * nn : c * 1024 + 512 * (nn + 1)],
                        start=(c == 0),
                        stop=(c == 1),
                    )
        u0_sb = st.tile([128, 2048], f16, tag="u0_sb")
        for j in range(2):
            nc.scalar.activation(u0_sb[:, H1024[j]], u0_ps[:, H1024[j]], AF.Identity)

        # u0_nm [n, (cp, h, tl')]: strided read, contiguous write
        u0_nm = st.tile([128, 2048], f16, tag="u0_nm")
        uiv = u0_sb[:].rearrange("p (cp nl h) -> p cp h nl", cp=2, nl=32, h=32)
        for cp in range(2):
            nc.vector.transpose(out=u0_nm[:, H1024[cp]], in_=uiv[:, cp])

        # z1 = tanh(P w0 + Q u0)  -> NM' [n, (c, h, tl)]
        # P moving: w0_nm viewed (c, h, tl) = strided-inner; Q moving: contiguous
        zpre0 = ps.tile([128, 2048], fp, tag="big")
        dummy(zpre0[:, C512[0]], u0_sb)
        w0v = w0_nm[:].rearrange("p (c tl h) -> p c h tl", c=2, tl=32, h=32)
        for c in range(2):
            for hh in range(2):
                j = 2 * c + hh
                mm(zpre0[:, C512[j]], pmat, w0v[:, c, 16 * hh : 16 * (hh + 1), :],
                   start=True, stop=False)
        for j in range(4):
            mm(zpre0[:, C512[j]], qmat, u0_nm[:, C512[j]], start=False, stop=True)
        z1_nm = st.tile([128, 2048], f16, tag="z1_nm")
        for j in range(2):
            nc.scalar.activation(z1_nm[:, H1024[j]], zpre0[:, H1024[j]], AF.Tanh)

        # ====================== layer 2 (t-quarter only) ======================
        # g1 = FDT of z1 [tl-part, (c, h, nl)]: contiguous both sides
        g1 = st.tile([128, 2048], f16, tag="g1")
        u1_ps = ps.tile([128, 1024], fp, tag="big")
        dummy(u1_ps[:, 0:512], z1_nm)
        for j in range(2):
            nc.vector.transpose(out=g1[:, H1024[j]], in_=z1_nm[:, H1024[j]])

        # u1 = At[quarter,:]-mix(z1): PSUM-accum over c -> free (h, nl), part (nh, tq)
        dummy(u1_ps[:, 0:512], g1)
        g1r = g1[:].rearrange("p (c h nl) -> p c h nl", c=2, h=32, nl=32)
        for c in range(2):
            for hh in range(2):
                mm(
                    u1_ps[:, 512 * hh : 512 * (hh + 1)],
                    blk(_ATBQ + c),
                    g1r[:, c, 16 * hh : 16 * (hh + 1), :],
                    start=(c == 0),
                    stop=(c == 1),
                )
        u1_sb = st.tile([128, 1024], f16, tag="u1_sb")
        nc.scalar.activation(u1_sb[:], u1_ps[:], AF.Identity)

        # u1_nm [n, (h, tq32)]: contiguous both sides
        u1_nm = st.tile([128, 1024], f16, tag="u1_nm")
        nc.vector.transpose(out=u1_nm[:], in_=u1_sb[:])

        # v = P z1[quarter] + Q u1  -> NM quarter, free (tq, h)
        vpre = ps.tile([128, 512], fp, tag="big")
        dummy(vpre[:], u1_sb)
        z1v = z1_nm[:].rearrange("p (c h tl) -> p c tl h", c=2, h=32, tl=32)
        u1v = u1_nm[:].rearrange("p (h t) -> p t h", h=32, t=32)
        mm(vpre[:], pmat, z1v[:, ds(cq, 1), ds(tl0, TQ), :], start=True, stop=False)
        mm(vpre[:], qmat, u1v[:, 0:TQ, :], start=False, stop=True)
        v_sb = st.tile([128, 512], f16, tag="v_sb")
        nc.scalar.activation(v_sb[:], vpre[:], AF.Identity)

        # v_fd [h-part, (tq, nl)]: contiguous both sides
        v_fd = st.tile([128, 512], f16, tag="v_fd")
        nc.vector.transpose(out=v_fd[:], in_=v_sb[:])

        # z2 = tanh(v @ H1')  (FD); out = z2 @ W2' + b2  (FD)
        h2pre = ps.tile([128, 512], fp, tag="big")
        dummy(h2pre[:], v_sb)
        mm(h2pre[:], h1m, v_fd[:], start=True, stop=True)
        z2_fd = st.tile([128, 512], f16, tag="z2_fd")
        nc.scalar.activation(z2_fd[:], h2pre[:], AF.Tanh)

        opre = ps.tile([128, 512], fp, tag="big")
        mm(opre[:], w2m, z2_fd[:], start=True, stop=True)
        out_fd = st.tile([128, 512], fp, tag="out_fd")
        nc.scalar.activation(out_fd[:], opre[:], AF.Identity, bias=b2t)

        # store in FD layout; the host unscrambles
        nc.sync.dma_start(outfd_d.ap(), out_fd[:])

    nc.compile()
    return nc


def _host_weights(Adj_t, Adj_s, s, H, W1, b1, W2, b2):
    f4 = np.float32
    I4 = np.eye(4, dtype=f4)
    I128 = np.eye(128, dtype=f4)
    Heff = H.sum(axis=1).astype(f4)  # [2, 32, 32]

    P = (s[0] * I128 + s[1] * Adj_s).astype(f4)
    Q = (s[2] * I128 + s[3] * Adj_s).astype(f4)

    W1p = (W1 @ Heff[0]).astype(f4)  # H-first: fold Heff0 into W1
    b1p = (b1 @ Heff[0]).astype(f4)
    w2pad = np.zeros((32, 32), dtype=f4)
    w2pad[:, :FOUT] = W2

    cst = np.zeros((NQ, 128, _NCBLK * 128), dtype=np.float16)
    for q in range(NQ):
        c = cst[q]
        c[:, 0:128] = np.kron(I4, W1p)
        c[:, 128:256] = P
        c[:, 256:384] = Q
        c[:, 384:512] = np.kron(I4, Heff[1])
        c[:, 512:640] = np.kron(I4, w2pad)
        for cc in range(2):
            for cp in range(2):
                i = _ATBD + 2 * cc + cp
                c[:, 128 * i : 128 * (i + 1)] = np.kron(
                    I4, Adj_t[32 * cc : 32 * (cc + 1), 32 * cp : 32 * (cp + 1)].astype(f4)
                )
        for cc in range(2):
            bq = np.zeros((32, 32), dtype=f4)
            bq[:, :TQ] = Adj_t[32 * cc : 32 * (cc + 1), TQ * q : TQ * (q + 1)]
            i = _ATBQ + cc
            c[:, 128 * i : 128 * (i + 1)] = np.kron(I4, bq)

    bias = np.zeros((128, 2), dtype=f4)
    bias[:, 0] = np.tile(b1p, 4)
    b2pad = np.zeros(32, dtype=f4)
    b2pad[:FOUT] = b2
    bias[:, 1] = np.tile(b2pad, 4)
    return cst, bias


def _in_maps(inputs):
    f4 = np.float32
    x = np.asarray(inputs["x"], dtype=f4)
    cst, bias = _host_weights(
        np.asarray(inputs["Adj_t"], dtype=f4),
        np.asarray(inputs["Adj_s"], dtype=f4),
        np.asarray(inputs["s"], dtype=f4),
        np.asarray(inputs["H"], dtype=f4),
        np.asarray(inputs["W1"], dtype=f4),
        np.asarray(inputs["b1"], dtype=f4),
        np.asarray(inputs["W2"], dtype=f4),
        np.asarray(inputs["b2"], dtype=f4),
    )
    # FD-marshalled x per batch: xfd[32*nh + f, 32*t + nl] = x[b, 128*t + 32*nh + nl, f]
    xfd = [
        np.ascontiguousarray(
            x[b].reshape(T, 4, 32, FIN).transpose(1, 3, 0, 2).reshape(128, 2048)
        ).astype(np.float16)
        for b in range(B)
    ]
    maps = []
    for c in range(NCORES):
        b, q = c // NQ, c % NQ
        maps.append(
            {"xfd": xfd[b], "cst": np.ascontiguousarray(cst[q]), "bias": bias}
        )
    return maps


def kernel(**inputs) -> np.ndarray:
    import os

    from concourse import bass_utils

    if "nc" not in _CACHE:
        _CACHE["nc"] = _build_nc()
    nc = _CACHE["nc"]

    maps = _in_maps(inputs)

    trace = bool(int(os.environ.get("GTCNN_TRACE", "0")))
    res = bass_utils.run_bass_kernel_spmd(
        nc,
        maps,
        core_ids=list(range(NCORES)),
        trace=trace,
        trace_cores=list(range(NCORES)) if trace else None,
        stitch_traces=False,
    )
    _CACHE["last_results"] = res

    out = np.empty((B, M, FOUT), dtype=np.float32)
    for c in range(NCORES):
        b, q = c // NQ, c % NQ
        arr = np.asarray(res.results[c]["outfd"]).reshape(4, 32, TQ, 32)
        out[b, 2048 * q : 2048 * (q + 1), :] = (
            arr[:, :FOUT, :, :].transpose(2, 0, 3, 1).reshape(2048, FOUT)
        )
    return out


# revision 21
# speedup vs baseline: 1.7021x; 1.0079x over previous
"""Trainium2 Bass kernel for nn_GTCNN (product-graph GTCNN, 2 layers, K collapsed).

Math (per batch b, x: [M=8192, 32]):
  Adj = s0*I + s1*kron(I_t, As) + s2*kron(At, I_s) + s3*kron(At, As),  T=64, N=128
  h0 = x @ W1 + b1
  h_{l+1} = tanh((Adj @ h_l) @ Heff_l),   Heff_l = sum_k H[l, k]   (einsum collapses k)
  out = h2 @ W2 + b2

Device algorithm (the feature mix commutes with the node mixes):
  layer 1 "H-first", Heff0 folded into W1 on the host:
    w0 = x @ (W1 Heff0) + b1 Heff0
    z1 = tanh(P w0 + Q At w0)          P = s0*I + s1*As, Q = s2*I + s3*As
  layer 2 "H-last", quarter only:
    v  = P z1 + Q At z1                (t-quarter rows)
    out = tanh(v Heff1) @ W2 + b2
  (At, As symmetric -> they serve directly as matmul stationaries.)

Sharding: core c -> (b = c // 4, t-quarter q = c % 4). Layer 1 computed fully per
b (4x redundant, no collectives); layer 2 + output restricted to the 16-t quarter.

Layouts (n = 32*nh + nl, t = 32*c + tl, partition-block always nh):
  FD   [32*nh + h,  (c, tl, nl)]    feature-on-partition (W1/Heff/W2 matmuls)
  NM   [n, (c, tl, h)]              node-on-partition, t-major (P/Q "w" side)
  NM'  [n, (c, h, tl)]              node-on-partition, h-major (P/Q result, z1)
  FDT  [32*nh + tl, (c, h-or-nl, ...)]  t-on-partition (At matmuls)
All layout moves are DVE 32x32 StreamTranspose with CONTIGUOUS writes (strided
writes measured 4.6 ns/elem vs 1.17 contiguous); the unavoidable strided side
is always a read (1.8x) or a strided-inner matmul moving operand.

Perf design:
  - fp16 on-chip (1 cycle/col matmuls, fast weight load); PSUM stays fp32;
    rel err ~1e-3, tolerance is 2e-2
  - every PSUM->SBUF crossing is one ACT op (bias / copy / tanh) doing the
    fp32->fp16 conversion
  - x pre-marshalled on host into FD (one contiguous DMA); out stored FD
  - 8 warmup matmuls + dummy matmuls with data deps on mid-pipeline tiles
    keep the PE busy so the HAM clock gate never re-throttles to 1.2 GHz;
    dummies write (start=stop=True) into PSUM regions whose real writers
    also open with start=True, so they are overwritten harmlessly
"""

import numpy as np

T, NS, B, FIN, HID, FOUT = 64, 128, 2, 32, 32, 16
M = T * NS
NCORES, NQ = 8, 4
TQ = T // NQ  # 16 t's per quarter

_CACHE = {}

# column-block offsets (x128) inside the packed const tensor
_W1, _P, _Q, _H1, _W2 = 0, 1, 2, 3, 4
_ATBD = 5  # 4 blocks: 2*c + cp
_ATBQ = 9  # 2 blocks: c
_NCBLK = 11


def _build_nc():
    from contextlib import ExitStack

    import concourse.mybir as mybir
    import concourse.tile as tile
    from concourse import bacc
    from concourse.bass import ds

    fp = mybir.dt.float32
    f16 = mybir.dt.float16
    AF = mybir.ActivationFunctionType

    nc = bacc.Bacc(
        "TRN2",
        target_bir_lowering=False,
        debug=False,
        enable_asserts=False,
        num_devices=NCORES,
    )

    xfd_d = nc.dram_tensor("xfd", [128, 2048], f16, kind="ExternalInput")
    cst_d = nc.dram_tensor("cst", [128, _NCBLK * 128], f16, kind="ExternalInput")
    bias_d = nc.dram_tensor("bias", [128, 2], fp, kind="ExternalInput")
    outfd_d = nc.dram_tensor("outfd", [128, 512], fp, kind="ExternalOutput")

    C512 = [slice(512 * j, 512 * (j + 1)) for j in range(4)]
    H1024 = [slice(1024 * j, 1024 * (j + 1)) for j in range(2)]

    with tile.TileContext(nc) as tc, ExitStack() as ctx:
        const = ctx.enter_context(tc.tile_pool(name="const", bufs=1))
        st = ctx.enter_context(tc.tile_pool(name="st", bufs=1))
        ps = ctx.enter_context(tc.tile_pool(name="ps", bufs=2, space="PSUM"))

        pid = nc.tensor.partition_id()  # on PE: consumed by the vpre matmul AP
        cq = (pid % NQ) // 2  # which t-half holds this core's quarter
        tl0 = (pid % 2) * TQ  # tl offset inside that half

        # ---- PE warmup scratch (vector memset: fast, DVE idle this early) ----
        warm_sb = st.tile([128, 512], f16, tag="warm_sb")
        nc.vector.memset(warm_sb[:], 0.0)
        # Preload the tanh activation table off the critical path: without
        # this the first real ACTIVATE pays a lazy 1.3us ACT_TABLE_LOAD.
        tblw = st.tile([128, 1], fp, tag="tblw")
        nc.scalar.activation(tblw[:], warm_sb[:, 0:1], AF.Tanh)

        # ---- loads, chunked so w0pre's first matmuls start before the full
        # transfers land: W1'/P/Q first, then x in halves, then the rest ----
        cs = const.tile([128, _NCBLK * 128], f16, tag="cs")
        x_fd = st.tile([128, 2048], f16, tag="x_fd")
        bt = const.tile([128, 2], fp, tag="bt")
        nc.sync.dma_start(cs[:, 0:384], cst_d.ap()[:, 0:384])
        nc.sync.dma_start(x_fd[:, 0:1024], xfd_d.ap()[:, 0:1024])
        nc.sync.dma_start(x_fd[:, 1024:2048], xfd_d.ap()[:, 1024:2048])
        nc.sync.dma_start(cs[:, 384:], cst_d.ap()[:, 384:])
        nc.sync.dma_start(bt[:], bias_d.ap())

        def blk(i):
            return cs[:, 128 * i : 128 * (i + 1)]

        w1m, pmat, qmat, h1m, w2m = blk(_W1), blk(_P), blk(_Q), blk(_H1), blk(_W2)
        b1t = bt[:, 0:1]
        b2t = bt[:, 1:2]
        mm = nc.tensor.matmul

        def dummy(out, src):
            """Keep-warm matmul: garbage into a region whose real writer
            opens with start=True. Fires when `src` (SBUF fp16) is ready."""
            mm(out, src[:, 0:128], src[:, 0:512], start=True, stop=True,
               skip_group_check=True)

        # ---- PE warmup: a few matmuls start the HAM clock ramp while the
        # DMAs land. They write the second psum buffer (u0_ps's region), so
        # w0pre has no WAW dependence on them and starts as soon as the DMA
        # semaphores are visible.
        w0pre = ps.tile([128, 2048], fp, tag="big")
        u0_ps = ps.tile([128, 2048], fp, tag="big")
        for _ in range(3):
            dummy(u0_ps[:, C512[0]], warm_sb)

        # =========================== layer 1 (full) ===========================
        # w0 = x @ W1' + b1'   -> FD [h-part, (c, tl, nl)]
        for j in range(4):
            mm(w0pre[:, C512[j]], w1m, x_fd[:, C512[j]], start=True, stop=True)
        w0_fd = st.tile([128, 2048], f16, tag="w0_fd")
        for j in range(2):
            nc.scalar.activation(w0_fd[:, H1024[j]], w0pre[:, H1024[j]], AF.Identity, bias=b1t)

        # g0 = FDT of w0 [tl-part, (c, nl, h)]: strided read, contiguous write
        g0 = st.tile([128, 2048], f16, tag="g0")
        gi = w0_fd[:].rearrange("p (c tl nl) -> p c nl tl", c=2, tl=32, nl=32)
        go = g0[:].rearrange("p (c nl h) -> p c nl h", c=2, nl=32, h=32)
        dummy(u0_ps[:, C512[0]], w0_fd)  # fires mid ACT/transpose phase
        for c in range(2):
            nc.vector.transpose(out=go[:, c], in_=gi[:, c])

        # u0 = At-mix(w0): FDT, PSUM-accum over c -> free (cp, nl, h).
        # Moving operands are plain contiguous 512-slices of g0 (nl-halves):
        # strided/3D moving APs measured 4 cycles/col vs 1 contiguous.
        # c outermost: all 4 start-matmuls depend only on g0's first t-half,
        # so they overlap the transpose of the second half.
        dummy(u0_ps[:, C512[0]], g0)
        for c in range(2):
            for cp in range(2):
                for nn in range(2):
                    mm(
                        u0_ps[:, cp * 1024 + 512 * nn : cp * 1024 + 512 * (nn + 1)],
                        blk(_ATBD + 2 * c + cp),
                        g0[:, c * 1024 + 512 * nn : c * 1024 + 512 * (nn + 1)],
                        start=(c == 0),
                        stop=(c == 1),
                    )
        # w0_nm [n, (c, tl, h)], contiguous both sides. Emitted after the u0
        # matmuls so the DVE scheduler keeps g0's two chunks back-to-back
        # (the u0 accumulation is the critical consumer).
        w0_nm = st.tile([128, 2048], f16, tag="w0_nm")
        for j in range(2):
            nc.vector.transpose(out=w0_nm[:, H1024[j]], in_=w0_fd[:, H1024[j]])

        u0_sb = st.tile([128, 2048], f16, tag="u0_sb")
        for j in range(2):
            nc.scalar.activation(u0_sb[:, H1024[j]], u0_ps[:, H1024[j]], AF.Identity)

        # u0_nm [n, (cp, h, tl')]: strided read, contiguous write
        u0_nm = st.tile([128, 2048], f16, tag="u0_nm")
        uiv = u0_sb[:].rearrange("p (cp nl h) -> p cp h nl", cp=2, nl=32, h=32)
        for cp in range(2):
            nc.vector.transpose(out=u0_nm[:, H1024[cp]], in_=uiv[:, cp])

        # z1 = tanh(P w0 + Q u0)  -> NM' [n, (c, h, tl)]
        # P moving: w0_nm viewed (c, h, tl) = strided-inner; Q moving: contiguous
        zpre0 = ps.tile([128, 2048], fp, tag="big")
        dummy(zpre0[:, C512[0]], u0_sb)
        w0v = w0_nm[:].rearrange("p (c tl h) -> p c h tl", c=2, tl=32, h=32)
        for c in range(2):
            for hh in range(2):
                j = 2 * c + hh
                mm(zpre0[:, C512[j]], pmat, w0v[:, c, 16 * hh : 16 * (hh + 1), :],
                   start=True, stop=False)
        for j in range(4):
            mm(zpre0[:, C512[j]], qmat, u0_nm[:, C512[j]], start=False, stop=True)
        z1_nm = st.tile([128, 2048], f16, tag="z1_nm")
        for j in range(2):
            nc.scalar.activation(z1_nm[:, H1024[j]], zpre0[:, H1024[j]], AF.Tanh)

        # ====================== layer 2 (t-quarter only) ======================
        # g1 = FDT of z1 [tl-part, (c, h, nl)]: contiguous both sides
        g1 = st.tile([128, 2048], f16, tag="g1")
        u1_ps = ps.tile([128, 1024], fp, tag="big")
        dummy(u1_ps[:, 0:512], z1_nm)
        for j in range(2):
            nc.vector.transpose(out=g1[:, H1024[j]], in_=z1_nm[:, H1024[j]])

        # vpre's P-half only needs z1, so it is emitted before the u1 group
        # and fills the PE while the g1 transposes run
        vpre = ps.tile([128, 512], fp, tag="big")
        z1v = z1_nm[:].rearrange("p (c h tl) -> p c tl h", c=2, h=32, tl=32)
        mm(vpre[:], pmat, z1v[:, ds(cq, 1), ds(tl0, TQ), :], start=True, stop=False)

        # u1 = At[quarter,:]-mix(z1): PSUM-accum over c -> free (h, nl), part (nh, tq)
        g1r = g1[:].rearrange("p (c h nl) -> p c h nl", c=2, h=32, nl=32)
        for c in range(2):
            for hh in range(2):
                mm(
                    u1_ps[:, 512 * hh : 512 * (hh + 1)],
                    blk(_ATBQ + c),
                    g1r[:, c, 16 * hh : 16 * (hh + 1), :],
                    start=(c == 0),
                    stop=(c == 1),
                )
        u1_sb = st.tile([128, 1024], f16, tag="u1_sb")
        nc.scalar.activation(u1_sb[:], u1_ps[:], AF.Identity)

        # u1_nm [n, (h, tq32)]: contiguous both sides
        u1_nm = st.tile([128, 1024], f16, tag="u1_nm")
        nc.vector.transpose(out=u1_nm[:], in_=u1_sb[:])

        # v = P z1[quarter] + Q u1  -> NM quarter, free (tq, h)
        u1v = u1_nm[:].rearrange("p (h t) -> p t h", h=32, t=32)
        mm(vpre[:], qmat, u1v[:, 0:TQ, :], start=False, stop=True)
        v_sb = st.tile([128, 512], f16, tag="v_sb")
        nc.scalar.activation(v_sb[:], vpre[:], AF.Identity)

        # v_fd [h-part, (tq, nl)]: contiguous both sides
        v_fd = st.tile([128, 512], f16, tag="v_fd")
        nc.vector.transpose(out=v_fd[:], in_=v_sb[:])

        # z2 = tanh(v @ H1')  (FD); out = z2 @ W2' + b2  (FD)
        h2pre = ps.tile([128, 512], fp, tag="big")
        dummy(h2pre[:], v_sb)
        mm(h2pre[:], h1m, v_fd[:], start=True, stop=True)
        z2_fd = st.tile([128, 512], f16, tag="z2_fd")
        nc.scalar.activation(z2_fd[:], h2pre[:], AF.Tanh)

        opre = ps.tile([128, 512], fp, tag="big")
        mm(opre[:], w2m, z2_fd[:], start=True, stop=True)
        out_fd = st.tile([128, 512], fp, tag="out_fd")
        nc.scalar.activation(out_fd[:], opre[:], AF.Identity, bias=b2t)

        # store in FD layout; the host unscrambles
        nc.sync.dma_start(outfd_d.ap(), out_fd[:])

    nc.compile()
    return nc


def _host_weights(Adj_t, Adj_s, s, H, W1, b1, W2, b2):
    f4 = np.float32
    I4 = np.eye(4, dtype=f4)
    I128 = np.eye(128, dtype=f4)
    Heff = H.sum(axis=1).astype(f4)  # [2, 32, 32]

    P = (s[0] * I128 + s[1] * Adj_s).astype(f4)
    Q = (s[2] * I128 + s[3] * Adj_s).astype(f4)

    W1p = (W1 @ Heff[0]).astype(f4)  # H-first: fold Heff0 into W1
    b1p = (b1 @ Heff[0]).astype(f4)
    w2pad = np.zeros((32, 32), dtype=f4)
    w2pad[:, :FOUT] = W2

    cst = np.zeros((NQ, 128, _NCBLK * 128), dtype=np.float16)
    for q in range(NQ):
        c = cst[q]
        c[:, 0:128] = np.kron(I4, W1p)
        c[:, 128:256] = P
        c[:, 256:384] = Q
        c[:, 384:512] = np.kron(I4, Heff[1])
        c[:, 512:640] = np.kron(I4, w2pad)
        for cc in range(2):
            for cp in range(2):
                i = _ATBD + 2 * cc + cp
                c[:, 128 * i : 128 * (i + 1)] = np.kron(
                    I4, Adj_t[32 * cc : 32 * (cc + 1), 32 * cp : 32 * (cp + 1)].astype(f4)
                )
        for cc in range(2):
            bq = np.zeros((32, 32), dtype=f4)
            bq[:, :TQ] = Adj_t[32 * cc : 32 * (cc + 1), TQ * q : TQ * (q + 1)]
            i = _ATBQ + cc
            c[:, 128 * i : 128 * (i + 1)] = np.kron(I4, bq)

    bias = np.zeros((128, 2), dtype=f4)
    bias[:, 0] = np.tile(b1p, 4)
    b2pad = np.zeros(32, dtype=f4)
    b2pad[:FOUT] = b2
    bias[:, 1] = np.tile(b2pad, 4)
    return cst, bias


def _in_maps(inputs):
    f4 = np.float32
    x = np.asarray(inputs["x"], dtype=f4)
    cst, bias = _host_weights(
        np.asarray(inputs["Adj_t"], dtype=f4),
        np.asarray(inputs["Adj_s"], dtype=f4),
        np.asarray(inputs["s"], dtype=f4),
        np.asarray(inputs["H"], dtype=f4),
        np.asarray(inputs["W1"], dtype=f4),
        np.asarray(inputs["b1"], dtype=f4),
        np.asarray(inputs["W2"], dtype=f4),
        np.asarray(inputs["b2"], dtype=f4),
    )
    # FD-marshalled x per batch: xfd[32*nh + f, 32*t + nl] = x[b, 128*t + 32*nh + nl, f]
    xfd = [
        np.ascontiguousarray(
            x[b].reshape(T, 4, 32, FIN).transpose(1, 3, 0, 2).reshape(128, 2048)
        ).astype(np.float16)
        for b in range(B)
    ]
    maps = []
    for c in range(NCORES):
        b, q = c // NQ, c % NQ
        maps.append(
            {"xfd": xfd[b], "cst": np.ascontiguousarray(cst[q]), "bias": bias}
        )
    return maps


def kernel(**inputs) -> np.ndarray:
    import os

    from concourse import bass_utils

    if "nc" not in _CACHE:
        _CACHE["nc"] = _build_nc()
    nc = _CACHE["nc"]

    maps = _in_maps(inputs)

    trace = bool(int(os.environ.get("GTCNN_TRACE", "0")))
    res = bass_utils.run_bass_kernel_spmd(
        nc,
        maps,
        core_ids=list(range(NCORES)),
        trace=trace,
        trace_cores=list(range(NCORES)) if trace else None,
        stitch_traces=False,
    )
    _CACHE["last_results"] = res

    out = np.empty((B, M, FOUT), dtype=np.float32)
    for c in range(NCORES):
        b, q = c // NQ, c % NQ
        arr = np.asarray(res.results[c]["outfd"]).reshape(4, 32, TQ, 32)
        out[b, 2048 * q : 2048 * (q + 1), :] = (
            arr[:, :FOUT, :, :].transpose(2, 0, 3, 1).reshape(2048, FOUT)
        )
    return out


# revision 23
# speedup vs baseline: 1.7255x; 1.0138x over previous
"""Trainium2 Bass kernel for nn_GTCNN (product-graph GTCNN, 2 layers, K collapsed).

Math (per batch b, x: [M=8192, 32]):
  Adj = s0*I + s1*kron(I_t, As) + s2*kron(At, I_s) + s3*kron(At, As),  T=64, N=128
  h0 = x @ W1 + b1
  h_{l+1} = tanh((Adj @ h_l) @ Heff_l),   Heff_l = sum_k H[l, k]   (einsum collapses k)
  out = h2 @ W2 + b2

Device algorithm (the feature mix commutes with the node mixes):
  layer 1 "H-first", Heff0 folded into W1 on the host:
    w0 = x @ (W1 Heff0) + b1 Heff0
    z1 = tanh(P w0 + Q At w0)          P = s0*I + s1*As, Q = s2*I + s3*As
  layer 2 "H-last", quarter only:
    v  = P z1 + Q At z1                (t-quarter rows)
    out = tanh(v Heff1) @ W2 + b2
  (At, As symmetric -> they serve directly as matmul stationaries.)

Sharding: core c -> (b = c // 4, t-quarter q = c % 4). Layer 1 computed fully per
b (4x redundant, no collectives); layer 2 + output restricted to the 16-t quarter.

Layouts (n = 32*nh + nl, t = 32*c + tl, partition-block always nh):
  FD   [32*nh + h,  (c, tl, nl)]    feature-on-partition (W1/Heff/W2 matmuls)
  NM   [n, (c, tl, h)]              node-on-partition, t-major (P/Q "w" side)
  NM'  [n, (c, h, tl)]              node-on-partition, h-major (P/Q result, z1)
  FDT  [32*nh + tl, (c, h-or-nl, ...)]  t-on-partition (At matmuls)
All layout moves are DVE 32x32 StreamTranspose with CONTIGUOUS writes (strided
writes measured 4.6 ns/elem vs 1.17 contiguous); the unavoidable strided side
is always a read (1.8x) or a strided-inner matmul moving operand.

Perf design:
  - fp16 on-chip (1 cycle/col matmuls, fast weight load); PSUM stays fp32;
    rel err ~1e-3, tolerance is 2e-2
  - every PSUM->SBUF crossing is one ACT op (bias / copy / tanh) doing the
    fp32->fp16 conversion
  - x pre-marshalled on host into FD (one contiguous DMA); out stored FD
  - 8 warmup matmuls + dummy matmuls with data deps on mid-pipeline tiles
    keep the PE busy so the HAM clock gate never re-throttles to 1.2 GHz;
    dummies write (start=stop=True) into PSUM regions whose real writers
    also open with start=True, so they are overwritten harmlessly
"""

import numpy as np

T, NS, B, FIN, HID, FOUT = 64, 128, 2, 32, 32, 16
M = T * NS
NCORES, NQ = 8, 4
TQ = T // NQ  # 16 t's per quarter

_CACHE = {}

# column-block offsets (x128) inside the packed const tensor
_W1, _P, _Q, _H1, _W2 = 0, 1, 2, 3, 4
_ATBD = 5  # 4 blocks: 2*c + cp
_ATBQ = 9  # 2 blocks: c
_NCBLK = 11


def _build_nc():
    from contextlib import ExitStack

    import concourse.mybir as mybir
    import concourse.tile as tile
    from concourse import bacc
    from concourse.bass import ds

    fp = mybir.dt.float32
    f16 = mybir.dt.float16
    AF = mybir.ActivationFunctionType

    nc = bacc.Bacc(
        "TRN2",
        target_bir_lowering=False,
        debug=False,
        enable_asserts=False,
        num_devices=NCORES,
    )

    xfd_d = nc.dram_tensor("xfd", [128, 2048], f16, kind="ExternalInput")
    cst_d = nc.dram_tensor("cst", [128, _NCBLK * 128], f16, kind="ExternalInput")
    bias_d = nc.dram_tensor("bias", [128, 2], fp, kind="ExternalInput")
    outfd_d = nc.dram_tensor("outfd", [128, 512], fp, kind="ExternalOutput")

    C512 = [slice(512 * j, 512 * (j + 1)) for j in range(4)]
    H1024 = [slice(1024 * j, 1024 * (j + 1)) for j in range(2)]

    with tile.TileContext(nc) as tc, ExitStack() as ctx:
        const = ctx.enter_context(tc.tile_pool(name="const", bufs=1))
        st = ctx.enter_context(tc.tile_pool(name="st", bufs=1))
        ps = ctx.enter_context(tc.tile_pool(name="ps", bufs=2, space="PSUM"))

        pid = nc.tensor.partition_id()  # on PE: consumed by the vpre matmul AP
        cq = (pid % NQ) // 2  # which t-half holds this core's quarter
        tl0 = (pid % 2) * TQ  # tl offset inside that half

        # ---- PE warmup scratch (vector memset: fast, DVE idle this early) ----
        warm_sb = st.tile([128, 512], f16, tag="warm_sb")
        nc.vector.memset(warm_sb[:], 0.0)
        # Preload the tanh activation table off the critical path: without
        # this the first real ACTIVATE pays a lazy 1.3us ACT_TABLE_LOAD.
        tblw = st.tile([128, 1], fp, tag="tblw")
        nc.scalar.activation(tblw[:], warm_sb[:, 0:1], AF.Tanh)

        # ---- loads, chunked so w0pre's first matmuls start before the full
        # transfers land: W1'/P/Q first, then x in halves, then the rest ----
        cs = const.tile([128, _NCBLK * 128], f16, tag="cs")
        x_fd = st.tile([128, 2048], f16, tag="x_fd")
        bt = const.tile([128, 2], fp, tag="bt")
        nc.sync.dma_start(cs[:, 0:384], cst_d.ap()[:, 0:384])
        nc.sync.dma_start(x_fd[:, 0:1024], xfd_d.ap()[:, 0:1024])
        nc.sync.dma_start(x_fd[:, 1024:2048], xfd_d.ap()[:, 1024:2048])
        nc.sync.dma_start(cs[:, 384:], cst_d.ap()[:, 384:])
        nc.sync.dma_start(bt[:], bias_d.ap())

        def blk(i):
            return cs[:, 128 * i : 128 * (i + 1)]

        w1m, pmat, qmat, h1m, w2m = blk(_W1), blk(_P), blk(_Q), blk(_H1), blk(_W2)
        b1t = bt[:, 0:1]
        b2t = bt[:, 1:2]
        mm = nc.tensor.matmul

        def dummy(out, src):
            """Keep-warm matmul: garbage into a region whose real writer
            opens with start=True. Fires when `src` (SBUF fp16) is ready."""
            mm(out, src[:, 0:128], src[:, 0:512], start=True, stop=True,
               skip_group_check=True)

        # ---- PE warmup: a few matmuls start the HAM clock ramp while the
        # DMAs land. They write the second psum buffer (u0_ps's region), so
        # w0pre has no WAW dependence on them and starts as soon as the DMA
        # semaphores are visible.
        w0pre = ps.tile([128, 2048], fp, tag="big")
        u0_ps = ps.tile([128, 2048], fp, tag="big")
        for _ in range(3):
            dummy(u0_ps[:, C512[0]], warm_sb)

        # =========================== layer 1 (full) ===========================
        # w0 = x @ W1' + b1'   -> FD [h-part, (c, tl, nl)]
        for j in range(4):
            mm(w0pre[:, C512[j]], w1m, x_fd[:, C512[j]], start=True, stop=True)
        w0_fd = st.tile([128, 2048], f16, tag="w0_fd")
        for j in range(4):
            nc.scalar.activation(w0_fd[:, C512[j]], w0pre[:, C512[j]], AF.Identity, bias=b1t)

        # g0 = FDT of w0 [tl-part, (c, nl, h)]: strided read, contiguous write
        g0 = st.tile([128, 2048], f16, tag="g0")
        gi = w0_fd[:].rearrange("p (c tl nl) -> p c nl tl", c=2, tl=32, nl=32)
        go = g0[:].rearrange("p (c nl h) -> p c nl h", c=2, nl=32, h=32)
        dummy(u0_ps[:, C512[0]], w0_fd)  # fires mid ACT/transpose phase
        for c in range(2):
            nc.vector.transpose(out=go[:, c], in_=gi[:, c])

        # u0 = At-mix(w0): FDT, PSUM-accum over c -> free (cp, nl, h).
        # Moving operands are plain contiguous 512-slices of g0 (nl-halves):
        # strided/3D moving APs measured 4 cycles/col vs 1 contiguous.
        # c outermost: all 4 start-matmuls depend only on g0's first t-half,
        # so they overlap the transpose of the second half.
        dummy(u0_ps[:, C512[0]], g0)
        for c in range(2):
            for cp in range(2):
                for nn in range(2):
                    mm(
                        u0_ps[:, cp * 1024 + 512 * nn : cp * 1024 + 512 * (nn + 1)],
                        blk(_ATBD + 2 * c + cp),
                        g0[:, c * 1024 + 512 * nn : c * 1024 + 512 * (nn + 1)],
                        start=(c == 0),
                        stop=(c == 1),
                    )
        # w0_nm [n, (c, tl, h)], contiguous both sides. Emitted after the u0
        # matmuls so the DVE scheduler keeps g0's two chunks back-to-back
        # (the u0 accumulation is the critical consumer).
        w0_nm = st.tile([128, 2048], f16, tag="w0_nm")
        for j in range(2):
            nc.vector.transpose(out=w0_nm[:, H1024[j]], in_=w0_fd[:, H1024[j]])

        u0_sb = st.tile([128, 2048], f16, tag="u0_sb")
        for j in range(2):
            nc.scalar.activation(u0_sb[:, H1024[j]], u0_ps[:, H1024[j]], AF.Identity)

        # u0_nm [n, (cp, h, tl')]: strided read, contiguous write
        u0_nm = st.tile([128, 2048], f16, tag="u0_nm")
        uiv = u0_sb[:].rearrange("p (cp nl h) -> p cp h nl", cp=2, nl=32, h=32)
        for cp in range(2):
            nc.vector.transpose(out=u0_nm[:, H1024[cp]], in_=uiv[:, cp])

        # z1 = tanh(P w0 + Q u0)  -> NM' [n, (c, h, tl)]
        # P moving: w0_nm viewed (c, h, tl) = strided-inner; Q moving: contiguous
        zpre0 = ps.tile([128, 2048], fp, tag="big")
        dummy(zpre0[:, C512[0]], u0_sb)
        w0v = w0_nm[:].rearrange("p (c tl h) -> p c h tl", c=2, tl=32, h=32)
        for c in range(2):
            for hh in range(2):
                j = 2 * c + hh
                mm(zpre0[:, C512[j]], pmat, w0v[:, c, 16 * hh : 16 * (hh + 1), :],
                   start=True, stop=False)
        for j in range(4):
            mm(zpre0[:, C512[j]], qmat, u0_nm[:, C512[j]], start=False, stop=True)
        z1_nm = st.tile([128, 2048], f16, tag="z1_nm")
        for j in range(2):
            nc.scalar.activation(z1_nm[:, H1024[j]], zpre0[:, H1024[j]], AF.Tanh)

        # ====================== layer 2 (t-quarter only) ======================
        # g1 = FDT of z1 [tl-part, (c, h, nl)]: contiguous both sides
        g1 = st.tile([128, 2048], f16, tag="g1")
        u1_ps = ps.tile([128, 1024], fp, tag="big")
        dummy(u1_ps[:, 0:512], z1_nm)
        for j in range(2):
            nc.vector.transpose(out=g1[:, H1024[j]], in_=z1_nm[:, H1024[j]])

        # vpre's P-half only needs z1, so it is emitted before the u1 group
        # and fills the PE while the g1 transposes run
        vpre = ps.tile([128, 512], fp, tag="big")
        z1v = z1_nm[:].rearrange("p (c h tl) -> p c tl h", c=2, h=32, tl=32)
        mm(vpre[:], pmat, z1v[:, ds(cq, 1), ds(tl0, TQ), :], start=True, stop=False)

        # u1 = At[quarter,:]-mix(z1): PSUM-accum over c -> free (h, nl), part (nh, tq)
        g1r = g1[:].rearrange("p (c h nl) -> p c h nl", c=2, h=32, nl=32)
        for c in range(2):
            for hh in range(2):
                mm(
                    u1_ps[:, 512 * hh : 512 * (hh + 1)],
                    blk(_ATBQ + c),
                    g1r[:, c, 16 * hh : 16 * (hh + 1), :],
                    start=(c == 0),
                    stop=(c == 1),
                )
        u1_sb = st.tile([128, 1024], f16, tag="u1_sb")
        nc.scalar.activation(u1_sb[:], u1_ps[:], AF.Identity)

        # u1_nm [n, (h, tq32)]: contiguous both sides
        u1_nm = st.tile([128, 1024], f16, tag="u1_nm")
        nc.vector.transpose(out=u1_nm[:], in_=u1_sb[:])

        # v = P z1[quarter] + Q u1  -> NM quarter, free (tq, h)
        u1v = u1_nm[:].rearrange("p (h t) -> p t h", h=32, t=32)
        mm(vpre[:], qmat, u1v[:, 0:TQ, :], start=False, stop=True)

        # tail: everything 2-way chunked (tq halves) so the six remaining
        # serial 512-wide ops overlap pairwise across ACT/DVE/PE
        Q256 = [slice(256 * j, 256 * (j + 1)) for j in range(2)]
        v_sb = st.tile([128, 512], f16, tag="v_sb")
        for j in range(2):
            nc.scalar.activation(v_sb[:, Q256[j]], vpre[:, Q256[j]], AF.Identity)

        # v_fd [h-part, (tq, nl)]: contiguous both sides
        v_fd = st.tile([128, 512], f16, tag="v_fd")
        h2pre = ps.tile([128, 512], fp, tag="big")
        dummy(h2pre[:], v_sb)
        for j in range(2):
            nc.vector.transpose(out=v_fd[:, Q256[j]], in_=v_sb[:, Q256[j]])

        # z2 = tanh(v @ H1')  (FD); out = z2 @ W2' + b2  (FD)
        z2_fd = st.tile([128, 512], f16, tag="z2_fd")
        for j in range(2):
            mm(h2pre[:, Q256[j]], h1m, v_fd[:, Q256[j]], start=True, stop=True)
        for j in range(2):
            nc.scalar.activation(z2_fd[:, Q256[j]], h2pre[:, Q256[j]], AF.Tanh)

        opre = ps.tile([128, 512], fp, tag="big")
        out_fd = st.tile([128, 512], fp, tag="out_fd")
        for j in range(2):
            mm(opre[:, Q256[j]], w2m, z2_fd[:, Q256[j]], start=True, stop=True)
        for j in range(2):
            nc.scalar.activation(out_fd[:, Q256[j]], opre[:, Q256[j]], AF.Identity, bias=b2t)

        # store in FD layout; the host unscrambles
        nc.sync.dma_start(outfd_d.ap(), out_fd[:])

    nc.compile()
    return nc


def _host_weights(Adj_t, Adj_s, s, H, W1, b1, W2, b2):
    f4 = np.float32
    I4 = np.eye(4, dtype=f4)
    I128 = np.eye(128, dtype=f4)
    Heff = H.sum(axis=1).astype(f4)  # [2, 32, 32]

    P = (s[0] * I128 + s[1] * Adj_s).astype(f4)
    Q = (s[2] * I128 + s[3] * Adj_s).astype(f4)

    W1p = (W1 @ Heff[0]).astype(f4)  # H-first: fold Heff0 into W1
    b1p = (b1 @ Heff[0]).astype(f4)
    w2pad = np.zeros((32, 32), dtype=f4)
    w2pad[:, :FOUT] = W2

    cst = np.zeros((NQ, 128, _NCBLK * 128), dtype=np.float16)
    for q in range(NQ):
        c = cst[q]
        c[:, 0:128] = np.kron(I4, W1p)
        c[:, 128:256] = P
        c[:, 256:384] = Q
        c[:, 384:512] = np.kron(I4, Heff[1])
        c[:, 512:640] = np.kron(I4, w2pad)
        for cc in range(2):
            for cp in range(2):
                i = _ATBD + 2 * cc + cp
                c[:, 128 * i : 128 * (i + 1)] = np.kron(
                    I4, Adj_t[32 * cc : 32 * (cc + 1), 32 * cp : 32 * (cp + 1)].astype(f4)
                )
        for cc in range(2):
            bq = np.zeros((32, 32), dtype=f4)
            bq[:, :TQ] = Adj_t[32 * cc : 32 * (cc + 1), TQ * q : TQ * (q + 1)]
            i = _ATBQ + cc
            c[:, 128 * i : 128 * (i + 1)] = np.kron(I4, bq)

    bias = np.zeros((128, 2), dtype=f4)
    bias[:, 0] = np.tile(b1p, 4)
    b2pad = np.zeros(32, dtype=f4)
    b2pad[:FOUT] = b2
    bias[:, 1] = np.tile(b2pad, 4)
    return cst, bias


def _in_maps(inputs):
    f4 = np.float32
    x = np.asarray(inputs["x"], dtype=f4)
    cst, bias = _host_weights(
        np.asarray(inputs["Adj_t"], dtype=f4),
        np.asarray(inputs["Adj_s"], dtype=f4),
        np.asarray(inputs["s"], dtype=f4),
        np.asarray(inputs["H"], dtype=f4),
        np.asarray(inputs["W1"], dtype=f4),
        np.asarray(inputs["b1"], dtype=f4),
        np.asarray(inputs["W2"], dtype=f4),
        np.asarray(inputs["b2"], dtype=f4),
    )
    # FD-marshalled x per batch: xfd[32*nh + f, 32*t + nl] = x[b, 128*t + 32*nh + nl, f]
    xfd = [
        np.ascontiguousarray(
            x[b].reshape(T, 4, 32, FIN).transpose(1, 3, 0, 2).reshape(128, 2048)
        ).astype(np.float16)
        for b in range(B)
    ]
    maps = []
    for c in range(NCORES):
        b, q = c // NQ, c % NQ
        maps.append(
            {"xfd": xfd[b], "cst": np.ascontiguousarray(cst[q]), "bias": bias}
        )
    return maps


def kernel(**inputs) -> np.ndarray:
    import os

    from concourse import bass_utils

    if "nc" not in _CACHE:
        _CACHE["nc"] = _build_nc()
    nc = _CACHE["nc"]

    maps = _in_maps(inputs)

    trace = bool(int(os.environ.get("GTCNN_TRACE", "0")))
    res = bass_utils.run_bass_kernel_spmd(
        nc,
        maps,
        core_ids=list(range(NCORES)),
        trace=trace,
        trace_cores=list(range(NCORES)) if trace else None,
        stitch_traces=False,
    )
    _CACHE["last_results"] = res

    out = np.empty((B, M, FOUT), dtype=np.float32)
    for c in range(NCORES):
        b, q = c // NQ, c % NQ
        arr = np.asarray(res.results[c]["outfd"]).reshape(4, 32, TQ, 32)
        out[b, 2048 * q : 2048 * (q + 1), :] = (
            arr[:, :FOUT, :, :].transpose(2, 0, 3, 1).reshape(2048, FOUT)
        )
    return out
